# revision 18
# baseline (speedup 1.0000x reference)
"""Trainium2 Bass kernel for nn_FMAPModelWarping (retrieval_knn).

Host side does only tiny index/weight precompute (affine grids, bilinear
taps, im2col of the 3-channel input). All FLOP-heavy work (convs, NxN
correlations, bidirectional softmax) runs on 8 NeuronCores.

Sharding: core k = 2*b + s handles sample b (of 4) and row-half s of the
3600x3600 correlation matrices. Per-core partial column stats are combined
on the host (exact: the kernel never shifts by any column statistic).

Math restructure (exact, no approximation):
  g[n] = 1/U_h[n],  res_sum[m] = O[m] / U_v[m],
  O[m] = sum_n g[n] * eh[n,m] * ev[n,m]
with U_h = rowsum(eh), U_v = colsum(ev), eh = exp(Mh), ev = exp(Mv).

v2 layout: features quantized to fp8e4m3 in a dual-plane [64, 2, n]
layout so the correlation matmuls run in DoubleRow perf mode; exp(Mh)
row-blocks are kept in SBUF (no HBM spill); phase R (row stats) and
phase F (m-outer O/U_v matvec accumulation) are interleaved so the
activation engine's exp stream overlaps the PE/DVE work; part of the
exp(Mv) field is computed on the vector engine with a Schraudolph
bit-trick exponential (the small relative noise cancels between O and
U_v, which consume the same ev values).
"""

import numpy as np

B, C_IN, H, W = 4, 3, 60, 60
HID, FEAT = 64, 128
N = H * W               # 3600
NCORES = 8
HALF = N // 2           # 1800 rows per core
NBLK = 128              # correlation row-block (partition dim)
NNB = 15                # row blocks per core (15*128 = 1920, rows padded)
NPAD = NNB * NBLK       # 1920
MT = 450                # m-tile width
N_MT = N // MT          # 8 m tiles
BANKW = 512             # fp32 elems per PSUM bank
HALFG = 1824            # gather count for A-warps (1800 pad to mult of 16)

# Schraudolph constants (bf16 target): i16 = rne(x*SA + SB); bits as bf16.
SA = 128.0 / float(np.log(2.0))
SB = 127.0 * 128.0 - 5.5 - 1.86   # -1.86 centers the measured +1% bias


# ----------------------------------------------------------------------------
# Host-side prep: exact reference semantics for grids / bilinear taps / rolls
# ----------------------------------------------------------------------------

def _affine_coords(theta2x3):
    """Pixel-space sample coords (x, y) for torch affine_grid+grid_sample
    (align_corners=False), shape [H, W] each."""
    xs = (2.0 * np.arange(W, dtype=np.float64) + 1.0) / W - 1.0
    ys = (2.0 * np.arange(H, dtype=np.float64) + 1.0) / H - 1.0
    gx, gy = np.meshgrid(xs, ys)           # gx[i,j]=xs[j], gy[i,j]=ys[i]
    t = theta2x3.astype(np.float64)
    cx = t[0, 0] * gx + t[0, 1] * gy + t[0, 2]
    cy = t[1, 0] * gx + t[1, 1] * gy + t[1, 2]
    px = (cx + 1.0) * W * 0.5 - 0.5
    py = (cy + 1.0) * H * 0.5 - 0.5
    return px, py


def _bilinear_sample_host(img, px, py):
    """img [C,H,W] float32, sample at (px,py) [H,W]; zeros padding.
    Mirrors reference grid_sample exactly."""
    x0 = np.floor(px); y0 = np.floor(py)
    wx1 = (px - x0); wx0 = 1.0 - wx1
    wy1 = (py - y0); wy0 = 1.0 - wy1
    out = np.zeros((img.shape[0],) + px.shape, np.float64)
    flat = img.reshape(img.shape[0], -1).astype(np.float64)
    for ix, iy, wt in ((x0, y0, wx0 * wy0), (x0 + 1, y0, wx1 * wy0),
                       (x0, y0 + 1, wx0 * wy1), (x0 + 1, y0 + 1, wx1 * wy1)):
        valid = (ix >= 0) & (ix < W) & (iy >= 0) & (iy < H)
        ii = np.clip(ix, 0, W - 1).astype(np.int64)
        jj = np.clip(iy, 0, H - 1).astype(np.int64)
        v = flat[:, (jj * W + ii).ravel()].reshape(out.shape)
        out += v * (wt * valid)[None]
    return out.astype(np.float32)


def _back_taps(theta2x3, u, v):
    """Tap indices/weights for grid_sample(y, grid(Bm)) composed with the
    inverse roll. Returns idx [4,3600] int (in-range), wt [4,3600] f32."""
    px, py = _affine_coords(theta2x3)
    ii = np.arange(H)[:, None]; jj = np.arange(W)[None, :]
    qi = (ii - u) % H; qj = (jj - v) % W
    xs = px[qi, qj].ravel(); ys = py[qi, qj].ravel()
    x0 = np.floor(xs); y0 = np.floor(ys)
    fx = xs - x0; fy = ys - y0
    idxs, wts = [], []
    for ix, iy, wt in ((x0, y0, (1 - fx) * (1 - fy)), (x0 + 1, y0, fx * (1 - fy)),
                       (x0, y0 + 1, (1 - fx) * fy), (x0 + 1, y0 + 1, fx * fy)):
        valid = (ix >= 0) & (ix < W) & (iy >= 0) & (iy < H)
        cii = np.clip(ix, 0, W - 1).astype(np.int64)
        cjj = np.clip(iy, 0, H - 1).astype(np.int64)
        idxs.append(cjj * W + cii)
        wts.append((wt * valid).astype(np.float32))
    return np.stack(idxs), np.stack(wts)


def _host_prep(inputs):
    """Build the 8 per-core device input dicts."""
    import ml_dtypes
    x_a = np.asarray(inputs["input_a"], np.float32)
    x_b = np.asarray(inputs["input_b"], np.float32)
    w1 = np.asarray(inputs["w1"], np.float32)
    b1 = np.asarray(inputs["b1"], np.float32)
    w2 = np.asarray(inputs["w2"], np.float32)
    b2 = np.asarray(inputs["b2"], np.float32)
    noise = np.asarray(inputs["noise"], np.float32)
    u_roll = np.asarray(inputs["u_roll"])
    v_roll = np.asarray(inputs["v_roll"])
    swap = np.asarray(inputs["swap"])

    w1mat = w1.reshape(HID, C_IN * 9)                  # [64, 27]
    w1dup = np.concatenate([w1mat.T, w1mat.T], axis=1).copy()   # [27, 128]
    b1dup = np.concatenate([b1, b1])[:, None].copy()            # [128, 1]
    w2mat = w2.reshape(FEAT, HID)                      # [128, 64]
    w2dupT = np.concatenate([w2mat.T, w2mat.T], axis=0).copy()  # [128, 128]
    b2col = b2[:, None].copy()                                   # [128, 1]

    eye = np.eye(3, dtype=np.float64)
    mask = np.array([[1., 1., 1.], [1., 1., 1.], [0., 0., 0.]])

    # per (warp, sample): X1 im2col [27,3600]; gather tables for the B-warps
    # (full pixel range) and per-half tables for the A-warps (only this
    # core's half of output pixels is ever used downstream).
    X1 = np.zeros((B, 4, C_IN * 9, N), np.float32)
    GIDX_B = np.zeros((B, 2, 2, 128, N // 16), np.int16)
    WBC_B = np.zeros((B, 2, 2, 128, N), np.float32)
    GIDX_A = np.zeros((B, 2, 2, 2, 128, HALFG // 16), np.int16)
    WBC_A = np.zeros((B, 2, 2, 2, 128, HALF), np.float32)
    for wrp in range(4):
        sw = int(swap[wrp]) == 1
        for b in range(B):
            fwd = eye + 0.05 * noise[wrp, b].astype(np.float64) * mask
            bwd = np.linalg.inv(fwd)
            A_ = bwd if sw else fwd
            Bm = fwd if sw else bwd
            u = int(u_roll[wrp, b]); v = int(v_roll[wrp, b])
            img = x_a[b] if wrp in (0, 2) else x_b[b]
            x_r = np.roll(np.roll(img, -u, axis=1), -v, axis=2)
            px, py = _affine_coords(np.asarray(A_)[:2])
            xw = _bilinear_sample_host(x_r, px, py)       # [3,60,60]
            # im2col, zero-pad SAME, k = c*9 + ky*3 + kx
            pad = np.zeros((C_IN, H + 2, W + 2), np.float32)
            pad[:, 1:-1, 1:-1] = xw
            k = 0
            for c in range(C_IN):
                for ky in range(3):
                    for kx in range(3):
                        X1[b, wrp, k] = pad[c, ky:ky + H, kx:kx + W].ravel()
                        k += 1
            idx, wt = _back_taps(np.asarray(Bm)[:2], u, v)
            if wrp in (1, 3):
                wb = wrp // 2
                for call in range(2):
                    for grp in range(8):
                        tap = call * 2 + (0 if grp < 4 else 1)
                        seg = idx[tap].reshape(N // 16, 16).T   # [16, 225]
                        GIDX_B[b, wb, call, grp * 16:(grp + 1) * 16] = seg.astype(np.int16)
                    WBC_B[b, wb, call, 0:64] = wt[call * 2][None]
                    WBC_B[b, wb, call, 64:128] = wt[call * 2 + 1][None]
            else:
                wa = wrp // 2
                for s in range(2):
                    n0 = s * HALF
                    for call in range(2):
                        for grp in range(8):
                            tap = call * 2 + (0 if grp < 4 else 1)
                            seg = np.zeros(HALFG, np.int64)
                            seg[:HALF] = idx[tap][n0:n0 + HALF]
                            seg = seg.reshape(HALFG // 16, 16).T
                            GIDX_A[b, s, wa, call, grp * 16:(grp + 1) * 16] = \
                                seg.astype(np.int16)
                        WBC_A[b, s, wa, call, 0:64] = wt[call * 2][n0:n0 + HALF][None]
                        WBC_A[b, s, wa, call, 64:128] = wt[call * 2 + 1][n0:n0 + HALF][None]

    # U_v matvec stationaries: ones, except block 14 masks the 120 pad rows
    onesmask = np.ones((128, NNB), np.float32)
    onesmask[8:, NNB - 1] = 0.0

    in_maps = []
    for core in range(NCORES):
        b = core // 2
        s = core % 2
        in_maps.append({
            "x1_in": X1[b].astype(ml_dtypes.bfloat16),
            "w1dup_in": w1dup.astype(ml_dtypes.bfloat16),
            "b1dup_in": b1dup,
            "gidxa_in": GIDX_A[b, s],
            "wbca_in": WBC_A[b, s].astype(ml_dtypes.bfloat16),
            "gidxb_in": GIDX_B[b],
            "wbcb_in": WBC_B[b].astype(ml_dtypes.bfloat16),
            "w2dupT_in": w2dupT,
            "b2_in": b2col,
            "onesmask_in": onesmask.astype(ml_dtypes.bfloat16),
        })
    return in_maps


# ----------------------------------------------------------------------------
# Device kernel builder
# ----------------------------------------------------------------------------

_CACHED = {}

# m-tile groups sharing the PSUM accumulator banks (3 rows at partition
# offsets 0/32/64 per bank; O and U_v each get one bank).
SWEEPJS = [(0, 1, 2), (3, 4, 5), (6, 7)]

# F-chunk ev-exponential engine assignment. Early sweeps mostly run while
# phase R still owns the activation engine -> Schraudolph on DVE; later
# sweeps run post-R -> exact exp on ACT.
EV_ENGINE = {}
for _j in range(N_MT):
    for _c in range(5):
        EV_ENGINE[(_j, _c)] = "dve" if _j < 4 else "act"


def _build(core_half):
    """Build the Bacc module (one NEFF shared by all 8 cores; each core's
    row-half is fully encoded in its host-built gather tables/inputs)."""
    import concourse.bacc as bacc_mod
    import concourse.mybir as mybir
    from concourse.tile import TileContext
    from contextlib import ExitStack

    dt = mybir.dt
    Alu = mybir.AluOpType
    Act = mybir.ActivationFunctionType
    DR = mybir.MatmulPerfMode.DoubleRow

    nc = bacc_mod.Bacc("TRN2", target_bir_lowering=False)

    x1_in = nc.dram_tensor("x1_in", [4, C_IN * 9, N], dt.bfloat16, kind="ExternalInput")
    w1dup_in = nc.dram_tensor("w1dup_in", [C_IN * 9, FEAT], dt.bfloat16, kind="ExternalInput")
    b1dup_in = nc.dram_tensor("b1dup_in", [FEAT, 1], dt.float32, kind="ExternalInput")
    gidxa_in = nc.dram_tensor("gidxa_in", [2, 2, FEAT, HALFG // 16], dt.int16, kind="ExternalInput")
    wbca_in = nc.dram_tensor("wbca_in", [2, 2, FEAT, HALF], dt.bfloat16, kind="ExternalInput")
    gidxb_in = nc.dram_tensor("gidxb_in", [2, 2, FEAT, N // 16], dt.int16, kind="ExternalInput")
    wbcb_in = nc.dram_tensor("wbcb_in", [2, 2, FEAT, N], dt.bfloat16, kind="ExternalInput")
    w2dupT_in = nc.dram_tensor("w2dupT_in", [FEAT, FEAT], dt.float32, kind="ExternalInput")
    b2_in = nc.dram_tensor("b2_in", [FEAT, 1], dt.float32, kind="ExternalInput")
    onesmask_in = nc.dram_tensor("onesmask_in", [FEAT, NNB], dt.bfloat16, kind="ExternalInput")

    o_out = nc.dram_tensor("o_out", [3, 3, MT], dt.float32, kind="ExternalOutput")
    uv_out = nc.dram_tensor("uv_out", [3, 3, MT], dt.float32, kind="ExternalOutput")

    with ExitStack() as ctx:
        tc = ctx.enter_context(TileContext(nc))

        const = ctx.enter_context(tc.tile_pool(name="const", bufs=1))
        w1dup_t = const.tile([C_IN * 9, FEAT], dt.bfloat16)
        b1dup_t = const.tile([FEAT, 1], dt.float32)
        w2dupT_t = const.tile([FEAT, FEAT], dt.float32)
        b2_t = const.tile([FEAT, 1], dt.float32)
        onesmask_t = const.tile([FEAT, NNB], dt.bfloat16)
        nc.sync.dma_start(w1dup_t[:], w1dup_in[:])
        nc.sync.dma_start(b1dup_t[:], b1dup_in[:])
        nc.sync.dma_start(w2dupT_t[:], w2dupT_in[:])
        nc.sync.dma_start(b2_t[:], b2_in[:])
        nc.sync.dma_start(onesmask_t[:], onesmask_in[:])

        w2r_t = const.tile([FEAT, FEAT], dt.bfloat16)
        nc.vector.tensor_copy(w2r_t[:], w2dupT_t[:])

        # fp8 dual-plane feature tiles: [64, 2, cols]; feature d lives at
        # (partition d%64... plane d//64). A-side padded to NPAD cols.
        fpool = ctx.enter_context(tc.tile_pool(name="feat", bufs=1))
        f8ah = fpool.tile([64, 2, NPAD], dt.float8e4, name="f8ah")
        f8av = fpool.tile([64, 2, NPAD], dt.float8e4, name="f8av")
        f8bh = fpool.tile([64, 2, N], dt.float8e4, name="f8bh")
        f8bv = fpool.tile([64, 2, N], dt.float8e4, name="f8bv")
        # zero the 120-row pad sliver of the A-side tiles
        nc.vector.memset(f8ah[:, :, HALF:NPAD], 0.0)
        nc.vector.memset(f8av[:, :, HALF:NPAD], 0.0)

        stat = ctx.enter_context(tc.tile_pool(name="stat", bufs=1))
        eh_all = stat.tile([NBLK, NNB * N], dt.bfloat16, name="eh_all")
        g_all = stat.tile([NBLK, NNB], dt.bfloat16, name="g_all")

        rs = ctx.enter_context(tc.tile_pool(name="rsmall", bufs=10))

        # PSUM: R-pool banks (3), then feature-conv pools (transient, 4),
        # then F-pool (3) + O/Uv accumulator banks (2) after convs close.
        rp = ctx.enter_context(tc.tile_pool(name="rpsum", bufs=1, space="PSUM"))

        # R chunking: m ranges as (start, n_tiles) with 450-wide tiles
        RCH = [(0, 3), (1350, 3), (2700, 2)]

        def emit_r_block(nb):
            nsl = slice(nb * NBLK, (nb + 1) * NBLK)
            uhp = []
            for ci, (m0, nt) in enumerate(RCH):
                rt = rp.tile([NBLK, 3, BANKW], dt.float32, tag="r",
                             name=f"r_{nb}_{ci}")
                for k in range(nt):
                    nc.tensor.matmul(rt[:, k, 0:MT], f8ah[:, :, nsl],
                                     f8bh[:, :, m0 + k * MT: m0 + (k + 1) * MT],
                                     start=True, stop=True, perf_mode=DR)
                ehv = eh_all[:, nb * N + m0: nb * N + m0 + nt * MT] \
                    .rearrange("p (c w) -> p c w", w=MT)
                uh = rs.tile([NBLK, 1], dt.float32, tag=f"uh{ci}",
                             name=f"uh_{nb}_{ci}")
                nc.scalar.activation(ehv, rt[:, 0:nt, 0:MT], Act.Exp,
                                     accum_out=uh[:])
                uhp.append(uh)
            ua = rs.tile([NBLK, 1], dt.float32, tag="ua", name=f"ua_{nb}")
            nc.vector.tensor_tensor(ua[:], uhp[0][:], uhp[1][:], Alu.add)
            nc.vector.tensor_tensor(ua[:], ua[:], uhp[2][:], Alu.add)
            gr = rs.tile([NBLK, 1], dt.float32, tag="gr", name=f"gr_{nb}")
            nc.vector.reciprocal(gr[:], ua[:])
            nc.vector.tensor_copy(g_all[:, nb:nb + 1], gr[:])
            if nb == NNB - 1:   # zero g on the 120 pad rows
                nc.vector.tensor_tensor(g_all[:, nb:nb + 1], g_all[:, nb:nb + 1],
                                        onesmask_t[:, nb:nb + 1], Alu.mult)

        # ---------------- feature stage helpers -------------------------
        def emit_conv1(wrp, wk, wkp, eng):
            x1_t = wk.tile([C_IN * 9, N], dt.bfloat16, tag="x1", bufs=1,
                           name=f"x1_{wrp}")
            nc.sync.dma_start(x1_t[:], x1_in[wrp])
            # fp32: ap_gather needs 4-byte elements (d * dtype_size % 4 == 0)
            y1 = wk.tile([FEAT, N], dt.float32, tag="y1", bufs=1,
                         name=f"y1_{wrp}")
            for hh in range(4):
                cps = wkp.tile([FEAT, 2, BANKW], dt.float32, tag="cps",
                               name=f"c1_{wrp}_{hh}")
                for j in range(2):
                    mt = hh * 2 + j
                    nc.tensor.matmul(cps[:, j, 0:MT], w1dup_t[:],
                                     x1_t[:, mt * MT:(mt + 1) * MT],
                                     start=True, stop=True)
                dst = y1[:, hh * 2 * MT:(hh + 1) * 2 * MT] \
                    .rearrange("p (c w) -> p c w", w=MT)
                if eng == "act":
                    nc.scalar.activation(dst, cps[:, :, 0:MT], Act.Relu,
                                         bias=b1dup_t[:])
                else:
                    nc.vector.tensor_scalar(dst, cps[:, :, 0:MT], b1dup_t[:],
                                            0.0, Alu.add, Alu.max)
            return y1

        def emit_gather(wrp, y1, wk, zw_eng="dve"):
            is_a = wrp in (0, 2)
            gidx_src = gidxa_in[wrp // 2] if is_a else gidxb_in[wrp // 2]
            wbc_src = wbca_in[wrp // 2] if is_a else wbcb_in[wrp // 2]
            n_g = HALFG if is_a else N
            n_p = HALF if is_a else N
            zw = []
            for call in range(2):
                gidx_t = wk.tile([FEAT, n_g // 16], dt.int16, tag="gidx",
                                 name=f"gi_{wrp}_{call}", bufs=2)
                nc.sync.dma_start(gidx_t[:], gidx_src[call])
                wbc_t = wk.tile([FEAT, n_p], dt.bfloat16, tag="wbc", bufs=2,
                                name=f"wb_{wrp}_{call}")
                nc.sync.dma_start(wbc_t[:], wbc_src[call])
                zw_t = wk.tile([FEAT, n_p], dt.bfloat16, tag=f"zw{call}", bufs=1,
                               name=f"zw_{wrp}_{call}")
                hg = n_g // 2
                hg -= hg % 16
                for hf, (g0, g1) in enumerate(((0, hg), (hg, n_g))):
                    z_t = wk.tile([FEAT, n_g - hg], dt.float32, tag="z", bufs=2,
                                  name=f"z_{wrp}_{call}_{hf}")
                    nc.gpsimd.ap_gather(z_t[:, 0:g1 - g0], y1[:],
                                        gidx_t[:, g0 // 16:g1 // 16],
                                        channels=FEAT, num_elems=N, d=1,
                                        num_idxs=g1 - g0)
                    p1 = min(g1, n_p)
                    if p1 <= g0:
                        continue
                    eng = nc.vector if zw_eng == "dve" else nc.gpsimd
                    eng.tensor_tensor(zw_t[:, g0:p1], z_t[:, 0:p1 - g0],
                                      wbc_t[:, g0:p1], Alu.mult)
                zw.append(zw_t)
            return zw

        def emit_conv2(wrp, zw, wk, wkp, eng):
            is_a = wrp in (0, 2)
            n_p = HALF if is_a else N
            f8tmp = wk.tile([FEAT, n_p], dt.float8e4, tag="f8tmp", bufs=2,
                            name=f"f8t_{wrp}")
            for hh in range(n_p // (2 * MT)):
                cps2 = wkp.tile([FEAT, 2, BANKW], dt.float32, tag="cps",
                                name=f"c2_{wrp}_{hh}")
                for j in range(2):
                    mt = hh * 2 + j
                    sl = slice(mt * MT, (mt + 1) * MT)
                    nc.tensor.matmul(cps2[:, j, 0:MT], w2r_t[:], zw[0][:, sl],
                                     start=True, stop=False)
                    nc.tensor.matmul(cps2[:, j, 0:MT], w2r_t[:], zw[1][:, sl],
                                     start=False, stop=True)
                dst = f8tmp[:, hh * 2 * MT:(hh + 1) * 2 * MT] \
                    .rearrange("p (c w) -> p c w", w=MT)
                if eng == "act":
                    nc.scalar.activation(dst, cps2[:, :, 0:MT], Act.Identity,
                                         bias=b2_t[:])
                else:
                    nc.vector.tensor_scalar(dst, cps2[:, :, 0:MT], b2_t[:],
                                            None, Alu.add)
            return f8tmp

        def emit_repack(wrp, f8tmp):
            is_a = wrp in (0, 2)
            n_p = HALF if is_a else N
            dstt = {0: f8ah, 2: f8av, 1: f8bh, 3: f8bv}[wrp]
            nc.sync.dma_start(dstt[:, 0, 0:n_p], f8tmp[0:64, :])
            nc.sync.dma_start(dstt[:, 1, 0:n_p], f8tmp[64:128, :])

        # ---------------- phase F chunk ---------------------------------
        # built lazily after conv pools close (PSUM bank reuse)
        fstate = {}

        JPOS = {}
        for _s, _js in enumerate(SWEEPJS):
            for _p, _jv in enumerate(_js):
                JPOS[_jv] = _p

        def emit_f_chunk(j, nbc):
            fp = fstate["fp"]; fwk = fstate["fwk"]
            oacc = fstate["oacc"]; uvacc = fstate["uvacc"]
            jj = JPOS[j]
            jsl = slice(j * MT, (j + 1) * MT)
            nbs = [3 * nbc + k for k in range(3)]
            ft = fp.tile([NBLK, 3, BANKW], dt.float32, tag="f",
                         name=f"f_{j}_{nbc}")
            for kk, nb in enumerate(nbs):
                nsl = slice(nb * NBLK, (nb + 1) * NBLK)
                nc.tensor.matmul(ft[:, kk, 0:MT], f8av[:, :, nsl],
                                 f8bv[:, :, jsl], start=True, stop=True,
                                 perf_mode=DR)
            ev_i = fwk.tile([NBLK, 3, MT], dt.int16, tag="ev", bufs=3,
                            name=f"ev_{j}_{nbc}")
            evb = ev_i[:].bitcast(dt.bfloat16)
            if EV_ENGINE[(j, nbc)] == "act":
                nc.scalar.activation(evb, ft[:, :, 0:MT], Act.Exp)
            else:
                nc.vector.tensor_scalar(ev_i[:], ft[:, :, 0:MT], SA, SB,
                                        Alu.mult, Alu.add)
            ehv = eh_all[:].rearrange("p (b m) -> p b m", b=NNB) \
                [:, nbs[0]:nbs[0] + 3, jsl]
            t_t = fwk.tile([NBLK, 3, MT], dt.bfloat16, tag="t", bufs=3,
                           name=f"t_{j}_{nbc}")
            nc.vector.tensor_tensor(t_t[:], ehv, evb, Alu.mult)
            orow = oacc[32 * jj:32 * jj + 1, 0:MT]
            uvrow = uvacc[32 * jj:32 * jj + 1, 0:MT]
            for kk, nb in enumerate(nbs):
                nc.tensor.matmul(orow, g_all[:, nb:nb + 1], t_t[:, kk, :],
                                 start=(nb == 0), stop=(nb == NNB - 1),
                                 skip_group_check=True)
                nc.tensor.matmul(uvrow, onesmask_t[:, nb:nb + 1],
                                 ev_i[:, kk, :].bitcast(dt.bfloat16),
                                 start=(nb == 0), stop=(nb == NNB - 1),
                                 skip_group_check=True)

        def emit_sweep_drain(s):
            oacc = fstate["oacc"]; uvacc = fstate["uvacc"]
            fwk = fstate["fwk"]
            osb = fwk.tile([96, BANKW], dt.float32, tag="osb", bufs=2,
                           name=f"osb_{s}")
            uvsb = fwk.tile([96, BANKW], dt.float32, tag="uvsb", bufs=2,
                            name=f"uvsb_{s}")
            nc.scalar.copy(osb[:], oacc[0:96, :])
            nc.scalar.copy(uvsb[:], uvacc[0:96, :])
            ov = osb[:].rearrange("(q t) m -> q t m", t=32)[:, 0, 0:MT]
            uvv = uvsb[:].rearrange("(q t) m -> q t m", t=32)[:, 0, 0:MT]
            nc.sync.dma_start(o_out[s], ov)
            nc.sync.dma_start(uv_out[s], uvv)

        # ---------------- schedule --------------------------------------
        with tc.tile_pool(name="fwork", bufs=2) as wk, \
             tc.tile_pool(name="fpsum1", bufs=2, space="PSUM") as wkp:
            # warps 1 and 0 fully before phase R (ACT idle here)
            y1b = emit_conv1(1, wk, wkp, "act")
            y1a = emit_conv1(0, wk, wkp, "act")
            zwb = emit_gather(1, y1b, wk)
            zwa = emit_gather(0, y1a, wk)
            f8t1 = emit_conv2(1, zwb, wk, wkp, "act")
            emit_repack(1, f8t1)
            f8t0 = emit_conv2(0, zwa, wk, wkp, "act")
            emit_repack(0, f8t0)

            # phase R blocks 0..5 with warp 3 / warp 2 stages woven in
            # (relu/drain on DVE so ACT stays on the exp stream)
            stg = {}
            emit_r_block(0)
            stg["y3"] = emit_conv1(3, wk, wkp, "dve")
            emit_r_block(1)
            stg["zw3"] = emit_gather(3, stg["y3"], wk, zw_eng="gpsimd")
            emit_r_block(2)
            stg["f83"] = emit_conv2(3, stg["zw3"], wk, wkp, "dve")
            emit_repack(3, stg["f83"])
            emit_r_block(3)
            stg["y2"] = emit_conv1(2, wk, wkp, "dve")
            emit_r_block(4)
            stg["zw2"] = emit_gather(2, stg["y2"], wk, zw_eng="gpsimd")
            emit_r_block(5)
            stg["f82"] = emit_conv2(2, stg["zw2"], wk, wkp, "dve")
            emit_repack(2, stg["f82"])

        # conv psum pool closed; open F psum pools
        fstate["fp"] = ctx.enter_context(
            tc.tile_pool(name="fpsum2", bufs=1, space="PSUM"))
        fstate["fwk"] = ctx.enter_context(tc.tile_pool(name="fwk", bufs=1))
        accp = ctx.enter_context(
            tc.tile_pool(name="accpsum", bufs=1, space="PSUM"))
        fstate["oacc"] = accp.tile([NBLK, BANKW], dt.float32, name="oacc")
        fstate["uvacc"] = accp.tile([NBLK, BANKW], dt.float32, name="uvacc")

        # remaining R blocks interleaved with sweep-0 F chunks (js 0-2).
        # F chunk (j, nbc) needs R block 3*nbc+2 and g for those blocks.
        nsw0 = len(SWEEPJS[0])
        fqueue = [(j, c) for c in range(5) for j in SWEEPJS[0]]
        emitted = 0
        for nb in range(6, NNB):
            emit_r_block(nb)
            ready = nsw0 * ((nb - 2) // 3 + 1) if nb >= 2 else 0
            while emitted < min(ready, len(fqueue)):
                emit_f_chunk(*fqueue[emitted])
                emitted += 1
        while emitted < len(fqueue):
            emit_f_chunk(*fqueue[emitted])
            emitted += 1
        emit_sweep_drain(0)
        for s in range(1, len(SWEEPJS)):
            for j, c in [(j, c) for c in range(5) for j in SWEEPJS[s]]:
                emit_f_chunk(j, c)
            emit_sweep_drain(s)

    nc.compile()
    return nc


def _get_nc(s):
    if s not in _CACHED:
        _CACHED[s] = _build(s)
    return _CACHED[s]


# ----------------------------------------------------------------------------
# Entry point
# ----------------------------------------------------------------------------

def kernel(**inputs):
    from concourse.bass_utils import run_bass_kernel_spmd

    in_maps = _host_prep(inputs)

    # One program for all 8 cores: the row-half each core handles is fully
    # encoded in its host-built gather tables.
    nc = _get_nc(0)
    last_err = None
    for attempt in range(3):
        try:
            r = run_bass_kernel_spmd(nc, in_maps, core_ids=list(range(NCORES)))
            break
        except Exception as e:  # transient NRT_EXEC_UNIT_UNRECOVERABLE wedges
            last_err = e
            import time
            time.sleep(10 * (attempt + 1))
    else:
        raise last_err
    results = r.results

    # host combine (exact)
    def _gather_m(arr):
        out = np.zeros(N, np.float64)
        for s, js in enumerate(SWEEPJS):
            for p, j in enumerate(js):
                out[j * MT:(j + 1) * MT] = arr[s, p].astype(np.float64)
        return out

    logs = np.zeros((B, N), np.float64)
    for b in range(B):
        r0, r1 = results[2 * b], results[2 * b + 1]
        O = _gather_m(r0["o_out"]) + _gather_m(r1["o_out"])
        uv = _gather_m(r0["uv_out"]) + _gather_m(r1["uv_out"])
        res_sum = O / uv
        logs[b] = np.log(res_sum + 1e-4)
    return np.float32(logs.mean())


# revision 26
# speedup vs baseline: 1.0407x; 1.0407x over previous
"""Trainium2 Bass kernel for nn_FMAPModelWarping (retrieval_knn).

Host side does only tiny index/weight precompute (affine grids, bilinear
taps, im2col of the 3-channel input). All FLOP-heavy work (convs, NxN
correlations, bidirectional softmax) runs on 8 NeuronCores.

Sharding: core k = 2*b + s handles sample b (of 4) and row-half s of the
3600x3600 correlation matrices. Per-core partial column stats are combined
on the host (exact: the kernel never shifts by any column statistic).

Math restructure (exact, no approximation):
  g[n] = 1/U_h[n],  res_sum[m] = O[m] / U_v[m],
  O[m] = sum_n g[n] * eh[n,m] * ev[n,m]
with U_h = rowsum(eh), U_v = colsum(ev), eh = exp(Mh), ev = exp(Mv).

v2 layout: features quantized to fp8e4m3 in a dual-plane [64, 2, n]
layout so the correlation matmuls run in DoubleRow perf mode; exp(Mh)
row-blocks are kept in SBUF (no HBM spill); phase R (row stats) and
phase F (m-outer O/U_v matvec accumulation) are interleaved so the
activation engine's exp stream overlaps the PE/DVE work; part of the
exp(Mv) field is computed on the vector engine with a Schraudolph
bit-trick exponential (the small relative noise cancels between O and
U_v, which consume the same ev values).
"""

import numpy as np

B, C_IN, H, W = 4, 3, 60, 60
HID, FEAT = 64, 128
N = H * W               # 3600
NCORES = 8
HALF = N // 2           # 1800 rows per core
NBLK = 128              # correlation row-block (partition dim)
NNB = 15                # row blocks per core (15*128 = 1920, rows padded)
NPAD = NNB * NBLK       # 1920
MT = 450                # m-tile width
N_MT = N // MT          # 8 m tiles
BANKW = 512             # fp32 elems per PSUM bank
HALFG = 1824            # gather count for A-warps (1800 pad to mult of 16)

# Schraudolph constants (bf16 target): i16 = rne(x*SA + SB); bits as bf16.
SA = 128.0 / float(np.log(2.0))
SB = 127.0 * 128.0 - 5.5 - 1.86   # -1.86 centers the measured +1% bias


# ----------------------------------------------------------------------------
# Host-side prep: exact reference semantics for grids / bilinear taps / rolls
# ----------------------------------------------------------------------------

def _affine_coords(theta2x3):
    """Pixel-space sample coords (x, y) for torch affine_grid+grid_sample
    (align_corners=False), shape [H, W] each."""
    xs = (2.0 * np.arange(W, dtype=np.float64) + 1.0) / W - 1.0
    ys = (2.0 * np.arange(H, dtype=np.float64) + 1.0) / H - 1.0
    gx, gy = np.meshgrid(xs, ys)           # gx[i,j]=xs[j], gy[i,j]=ys[i]
    t = theta2x3.astype(np.float64)
    cx = t[0, 0] * gx + t[0, 1] * gy + t[0, 2]
    cy = t[1, 0] * gx + t[1, 1] * gy + t[1, 2]
    px = (cx + 1.0) * W * 0.5 - 0.5
    py = (cy + 1.0) * H * 0.5 - 0.5
    return px, py


def _bilinear_sample_host(img, px, py):
    """img [C,H,W] float32, sample at (px,py) [H,W]; zeros padding.
    Mirrors reference grid_sample exactly."""
    x0 = np.floor(px); y0 = np.floor(py)
    wx1 = (px - x0); wx0 = 1.0 - wx1
    wy1 = (py - y0); wy0 = 1.0 - wy1
    out = np.zeros((img.shape[0],) + px.shape, np.float64)
    flat = img.reshape(img.shape[0], -1).astype(np.float64)
    for ix, iy, wt in ((x0, y0, wx0 * wy0), (x0 + 1, y0, wx1 * wy0),
                       (x0, y0 + 1, wx0 * wy1), (x0 + 1, y0 + 1, wx1 * wy1)):
        valid = (ix >= 0) & (ix < W) & (iy >= 0) & (iy < H)
        ii = np.clip(ix, 0, W - 1).astype(np.int64)
        jj = np.clip(iy, 0, H - 1).astype(np.int64)
        v = flat[:, (jj * W + ii).ravel()].reshape(out.shape)
        out += v * (wt * valid)[None]
    return out.astype(np.float32)


def _back_taps(theta2x3, u, v):
    """Tap indices/weights for grid_sample(y, grid(Bm)) composed with the
    inverse roll. Returns idx [4,3600] int (in-range), wt [4,3600] f32."""
    px, py = _affine_coords(theta2x3)
    ii = np.arange(H)[:, None]; jj = np.arange(W)[None, :]
    qi = (ii - u) % H; qj = (jj - v) % W
    xs = px[qi, qj].ravel(); ys = py[qi, qj].ravel()
    x0 = np.floor(xs); y0 = np.floor(ys)
    fx = xs - x0; fy = ys - y0
    idxs, wts = [], []
    for ix, iy, wt in ((x0, y0, (1 - fx) * (1 - fy)), (x0 + 1, y0, fx * (1 - fy)),
                       (x0, y0 + 1, (1 - fx) * fy), (x0 + 1, y0 + 1, fx * fy)):
        valid = (ix >= 0) & (ix < W) & (iy >= 0) & (iy < H)
        cii = np.clip(ix, 0, W - 1).astype(np.int64)
        cjj = np.clip(iy, 0, H - 1).astype(np.int64)
        idxs.append(cjj * W + cii)
        wts.append((wt * valid).astype(np.float32))
    return np.stack(idxs), np.stack(wts)


def _host_prep(inputs):
    """Build the 8 per-core device input dicts."""
    import ml_dtypes
    x_a = np.asarray(inputs["input_a"], np.float32)
    x_b = np.asarray(inputs["input_b"], np.float32)
    w1 = np.asarray(inputs["w1"], np.float32)
    b1 = np.asarray(inputs["b1"], np.float32)
    w2 = np.asarray(inputs["w2"], np.float32)
    b2 = np.asarray(inputs["b2"], np.float32)
    noise = np.asarray(inputs["noise"], np.float32)
    u_roll = np.asarray(inputs["u_roll"])
    v_roll = np.asarray(inputs["v_roll"])
    swap = np.asarray(inputs["swap"])

    w1mat = w1.reshape(HID, C_IN * 9)                  # [64, 27]
    w1dup = np.concatenate([w1mat.T, w1mat.T], axis=1).copy()   # [27, 128]
    b1dup = np.concatenate([b1, b1])[:, None].copy()            # [128, 1]
    w2mat = w2.reshape(FEAT, HID)                      # [128, 64]
    w2dupT = np.concatenate([w2mat.T, w2mat.T], axis=0).copy()  # [128, 128]
    b2col = b2[:, None].copy()                                   # [128, 1]

    eye = np.eye(3, dtype=np.float64)
    mask = np.array([[1., 1., 1.], [1., 1., 1.], [0., 0., 0.]])

    # per (warp, sample): X1 im2col [27,3600]; gather tables for the B-warps
    # (full pixel range) and per-half tables for the A-warps (only this
    # core's half of output pixels is ever used downstream).
    X1 = np.zeros((B, 4, C_IN * 9, N), np.float32)
    GIDX_B = np.zeros((B, 2, 2, 128, N // 16), np.int16)
    WBC_B = np.zeros((B, 2, 2, 128, N), np.float32)
    GIDX_A = np.zeros((B, 2, 2, 2, 128, HALFG // 16), np.int16)
    WBC_A = np.zeros((B, 2, 2, 2, 128, HALF), np.float32)
    for wrp in range(4):
        sw = int(swap[wrp]) == 1
        for b in range(B):
            fwd = eye + 0.05 * noise[wrp, b].astype(np.float64) * mask
            bwd = np.linalg.inv(fwd)
            A_ = bwd if sw else fwd
            Bm = fwd if sw else bwd
            u = int(u_roll[wrp, b]); v = int(v_roll[wrp, b])
            img = x_a[b] if wrp in (0, 2) else x_b[b]
            x_r = np.roll(np.roll(img, -u, axis=1), -v, axis=2)
            px, py = _affine_coords(np.asarray(A_)[:2])
            xw = _bilinear_sample_host(x_r, px, py)       # [3,60,60]
            # im2col, zero-pad SAME, k = c*9 + ky*3 + kx
            pad = np.zeros((C_IN, H + 2, W + 2), np.float32)
            pad[:, 1:-1, 1:-1] = xw
            k = 0
            for c in range(C_IN):
                for ky in range(3):
                    for kx in range(3):
                        X1[b, wrp, k] = pad[c, ky:ky + H, kx:kx + W].ravel()
                        k += 1
            idx, wt = _back_taps(np.asarray(Bm)[:2], u, v)
            if wrp in (1, 3):
                wb = wrp // 2
                for call in range(2):
                    for grp in range(8):
                        tap = call * 2 + (0 if grp < 4 else 1)
                        seg = idx[tap].reshape(N // 16, 16).T   # [16, 225]
                        GIDX_B[b, wb, call, grp * 16:(grp + 1) * 16] = seg.astype(np.int16)
                    WBC_B[b, wb, call, 0:64] = wt[call * 2][None]
                    WBC_B[b, wb, call, 64:128] = wt[call * 2 + 1][None]
            else:
                wa = wrp // 2
                for s in range(2):
                    n0 = s * HALF
                    for call in range(2):
                        for grp in range(8):
                            tap = call * 2 + (0 if grp < 4 else 1)
                            seg = np.zeros(HALFG, np.int64)
                            seg[:HALF] = idx[tap][n0:n0 + HALF]
                            seg = seg.reshape(HALFG // 16, 16).T
                            GIDX_A[b, s, wa, call, grp * 16:(grp + 1) * 16] = \
                                seg.astype(np.int16)
                        WBC_A[b, s, wa, call, 0:64] = wt[call * 2][n0:n0 + HALF][None]
                        WBC_A[b, s, wa, call, 64:128] = wt[call * 2 + 1][n0:n0 + HALF][None]

    # U_v matvec stationaries: ones, except block 14 masks the 120 pad rows
    onesmask = np.ones((128, NNB), np.float32)
    onesmask[8:, NNB - 1] = 0.0

    in_maps = []
    for core in range(NCORES):
        b = core // 2
        s = core % 2
        in_maps.append({
            "x1_in": X1[b].astype(ml_dtypes.bfloat16),
            "w1dup_in": w1dup.astype(ml_dtypes.bfloat16),
            "b1dup_in": b1dup,
            "gidxa_in": GIDX_A[b, s],
            "wbca_in": WBC_A[b, s].astype(ml_dtypes.bfloat16),
            "gidxb_in": GIDX_B[b],
            "wbcb_in": WBC_B[b].astype(ml_dtypes.bfloat16),
            "w2dupT_in": w2dupT,
            "b2_in": b2col,
            "onesmask_in": onesmask.astype(ml_dtypes.bfloat16),
        })
    return in_maps


# ----------------------------------------------------------------------------
# Device kernel builder
# ----------------------------------------------------------------------------

_CACHED = {}

# m-tile groups sharing the PSUM accumulator banks (3 rows at partition
# offsets 0/32/64 per bank; O and U_v each get one bank).
SWEEPJS = [(0, 1, 2), (3, 4, 5), (6, 7)]

# F-chunk ev-exponential engine assignment. Early sweeps mostly run while
# phase R still owns the activation engine -> Schraudolph on DVE; later
# sweeps run post-R -> exact exp on ACT.
EV_ENGINE = {}
for _j in range(N_MT):
    for _c in range(5):
        EV_ENGINE[(_j, _c)] = "dve" if _j < 4 else "act"


def _build(core_half):
    """Build the Bacc module (one NEFF shared by all 8 cores; each core's
    row-half is fully encoded in its host-built gather tables/inputs)."""
    import concourse.bacc as bacc_mod
    import concourse.mybir as mybir
    from concourse.tile import TileContext
    from contextlib import ExitStack

    dt = mybir.dt
    Alu = mybir.AluOpType
    Act = mybir.ActivationFunctionType
    DR = mybir.MatmulPerfMode.DoubleRow

    nc = bacc_mod.Bacc("TRN2", target_bir_lowering=False)

    x1_in = nc.dram_tensor("x1_in", [4, C_IN * 9, N], dt.bfloat16, kind="ExternalInput")
    w1dup_in = nc.dram_tensor("w1dup_in", [C_IN * 9, FEAT], dt.bfloat16, kind="ExternalInput")
    b1dup_in = nc.dram_tensor("b1dup_in", [FEAT, 1], dt.float32, kind="ExternalInput")
    gidxa_in = nc.dram_tensor("gidxa_in", [2, 2, FEAT, HALFG // 16], dt.int16, kind="ExternalInput")
    wbca_in = nc.dram_tensor("wbca_in", [2, 2, FEAT, HALF], dt.bfloat16, kind="ExternalInput")
    gidxb_in = nc.dram_tensor("gidxb_in", [2, 2, FEAT, N // 16], dt.int16, kind="ExternalInput")
    wbcb_in = nc.dram_tensor("wbcb_in", [2, 2, FEAT, N], dt.bfloat16, kind="ExternalInput")
    w2dupT_in = nc.dram_tensor("w2dupT_in", [FEAT, FEAT], dt.float32, kind="ExternalInput")
    b2_in = nc.dram_tensor("b2_in", [FEAT, 1], dt.float32, kind="ExternalInput")
    onesmask_in = nc.dram_tensor("onesmask_in", [FEAT, NNB], dt.bfloat16, kind="ExternalInput")

    o_out = nc.dram_tensor("o_out", [3, 3, MT], dt.float32, kind="ExternalOutput")
    uv_out = nc.dram_tensor("uv_out", [3, 3, MT], dt.float32, kind="ExternalOutput")

    with ExitStack() as ctx:
        tc = ctx.enter_context(TileContext(nc))

        const = ctx.enter_context(tc.tile_pool(name="const", bufs=1))
        w1dup_t = const.tile([C_IN * 9, FEAT], dt.bfloat16)
        b1dup_t = const.tile([FEAT, 1], dt.float32)
        w2dupT_t = const.tile([FEAT, FEAT], dt.float32)
        b2_t = const.tile([FEAT, 1], dt.float32)
        onesmask_t = const.tile([FEAT, NNB], dt.bfloat16)
        nc.sync.dma_start(w1dup_t[:], w1dup_in[:])
        nc.sync.dma_start(b1dup_t[:], b1dup_in[:])
        nc.sync.dma_start(w2dupT_t[:], w2dupT_in[:])
        nc.sync.dma_start(b2_t[:], b2_in[:])
        nc.sync.dma_start(onesmask_t[:], onesmask_in[:])

        w2r_t = const.tile([FEAT, FEAT], dt.bfloat16)
        nc.vector.tensor_copy(w2r_t[:], w2dupT_t[:])

        # fp8 dual-plane feature tiles: [64, 2, cols]; feature d lives at
        # (partition d%64... plane d//64). A-side padded to NPAD cols.
        fpool = ctx.enter_context(tc.tile_pool(name="feat", bufs=1))
        f8ah = fpool.tile([64, 2, NPAD], dt.float8e4, name="f8ah")
        f8av = fpool.tile([64, 2, NPAD], dt.float8e4, name="f8av")
        f8bh = fpool.tile([64, 2, N], dt.float8e4, name="f8bh")
        f8bv = fpool.tile([64, 2, N], dt.float8e4, name="f8bv")
        # zero the 120-row pad sliver of the A-side tiles
        nc.vector.memset(f8ah[:, :, HALF:NPAD], 0.0)
        nc.vector.memset(f8av[:, :, HALF:NPAD], 0.0)

        stat = ctx.enter_context(tc.tile_pool(name="stat", bufs=1))
        eh_all = stat.tile([NBLK, NNB * N], dt.bfloat16, name="eh_all")
        g_all = stat.tile([NBLK, NNB], dt.bfloat16, name="g_all")

        rs = ctx.enter_context(tc.tile_pool(name="rsmall", bufs=10))

        # Single PSUM pool, bank budget 8: tag "r" (3 banks, phase R chunks),
        # tag "f" (3 banks, F chunks + feature convs, time-disjoint), and two
        # accumulator banks. Post-R, F chunks ping-pong tags "f"/"r".
        pz = ctx.enter_context(tc.tile_pool(name="pz", bufs=1, space="PSUM"))
        fwkpool = ctx.enter_context(tc.tile_pool(name="fwkpool", bufs=1))

        # R chunking: m ranges as (start, n_tiles) with 450-wide tiles
        RCH = [(0, 3), (1350, 3), (2700, 2)]

        def emit_r_chunk(nb, ci):
            nsl = slice(nb * NBLK, (nb + 1) * NBLK)
            m0, nt = RCH[ci]
            rt = pz.tile([NBLK, 3, BANKW], dt.float32, tag="r",
                         name=f"r_{nb}_{ci}")
            for k in range(nt):
                nc.tensor.matmul(rt[:, k, 0:MT], f8ah[:, :, nsl],
                                 f8bh[:, :, m0 + k * MT: m0 + (k + 1) * MT],
                                 start=True, stop=True, perf_mode=DR)
            ehv = eh_all[:, nb * N + m0: nb * N + m0 + nt * MT] \
                .rearrange("p (c w) -> p c w", w=MT)
            uh = rs.tile([NBLK, 1], dt.float32, tag=f"uh{ci}",
                         name=f"uh_{nb}_{ci}")
            nc.scalar.activation(ehv, rt[:, 0:nt, 0:MT], Act.Exp,
                                 accum_out=uh[:])
            return uh

        def emit_r_gfin(nb, uhp):
            ua = rs.tile([NBLK, 1], dt.float32, tag="ua", name=f"ua_{nb}")
            nc.vector.tensor_tensor(ua[:], uhp[0][:], uhp[1][:], Alu.add)
            nc.vector.tensor_tensor(ua[:], ua[:], uhp[2][:], Alu.add)
            gr = rs.tile([NBLK, 1], dt.float32, tag="gr", name=f"gr_{nb}")
            nc.vector.reciprocal(gr[:], ua[:])
            nc.vector.tensor_copy(g_all[:, nb:nb + 1], gr[:])
            if nb == NNB - 1:   # zero g on the 120 pad rows
                nc.vector.tensor_tensor(g_all[:, nb:nb + 1], g_all[:, nb:nb + 1],
                                        onesmask_t[:, nb:nb + 1], Alu.mult)

        # ---------------- feature stage helpers -------------------------
        # ---------------- phase F chunk ---------------------------------
        # built lazily after conv pools close (PSUM bank reuse)
        fstate = {}

        JPOS = {}
        for _s, _js in enumerate(SWEEPJS):
            for _p, _jv in enumerate(_js):
                JPOS[_jv] = _p

        def emit_f_front(j, nbc, tag, ev_eng):
            fwk = fwkpool
            jsl = slice(j * MT, (j + 1) * MT)
            nbs = [3 * nbc + k for k in range(3)]
            ft = pz.tile([NBLK, 3, BANKW], dt.float32, tag=tag,
                         name=f"f_{j}_{nbc}")
            for kk, nb in enumerate(nbs):
                nsl = slice(nb * NBLK, (nb + 1) * NBLK)
                nc.tensor.matmul(ft[:, kk, 0:MT], f8av[:, :, nsl],
                                 f8bv[:, :, jsl], start=True, stop=True,
                                 perf_mode=DR)
            ev_i = fwk.tile([NBLK, 3, MT], dt.int16, tag="ev", bufs=2,
                            name=f"ev_{j}_{nbc}")
            evb = ev_i[:].bitcast(dt.bfloat16)
            if ev_eng == "act":
                nc.scalar.activation(evb, ft[:, :, 0:MT], Act.Exp)
            else:
                nc.vector.tensor_scalar(ev_i[:], ft[:, :, 0:MT], SA, SB,
                                        Alu.mult, Alu.add)
            ehv = eh_all[:].rearrange("p (b m) -> p b m", b=NNB) \
                [:, nbs[0]:nbs[0] + 3, jsl]
            t_t = fwk.tile([NBLK, 3, MT], dt.bfloat16, tag="t", bufs=2,
                           name=f"t_{j}_{nbc}")
            nc.vector.tensor_tensor(t_t[:], ehv, evb, Alu.mult)
            return (j, nbc, t_t, ev_i)

        def emit_f_mvs(front):
            j, nbc, t_t, ev_i = front
            jj = JPOS[j]
            nbs = [3 * nbc + k for k in range(3)]
            orow = fstate["oacc"][32 * jj:32 * jj + 1, 0:MT]
            uvrow = fstate["uvacc"][32 * jj:32 * jj + 1, 0:MT]
            for kk, nb in enumerate(nbs):
                nc.tensor.matmul(orow, g_all[:, nb:nb + 1], t_t[:, kk, :],
                                 start=(nb == 0), stop=(nb == NNB - 1),
                                 skip_group_check=True)
                nc.tensor.matmul(uvrow, onesmask_t[:, nb:nb + 1],
                                 ev_i[:, kk, :].bitcast(dt.bfloat16),
                                 start=(nb == 0), stop=(nb == NNB - 1),
                                 skip_group_check=True)

        def emit_sweep_drain(s):
            oacc = fstate["oacc"]; uvacc = fstate["uvacc"]
            osb = fwkpool.tile([96, MT], dt.float32, tag="osb", bufs=1,
                               name=f"osb_{s}")
            uvsb = fwkpool.tile([96, MT], dt.float32, tag="uvsb", bufs=1,
                                name=f"uvsb_{s}")
            nc.scalar.copy(osb[:], oacc[0:96, 0:MT])
            nc.scalar.copy(uvsb[:], uvacc[0:96, 0:MT])
            ov = osb[:].rearrange("(q t) m -> q t m", t=32)[:, 0, :]
            uvv = uvsb[:].rearrange("(q t) m -> q t m", t=32)[:, 0, :]
            nc.sync.dma_start(o_out[s], ov)
            nc.sync.dma_start(uv_out[s], uvv)

        # ---------------- feature-stage piece generator ------------------
        def gen_warp(wrp, wk, wkp, eng, zw_eng="dve"):
            y1 = emit_conv1_gen = None
            x1_t = wk.tile([C_IN * 9, N], dt.bfloat16, tag="x1", bufs=1,
                           name=f"gx1_{wrp}")
            nc.sync.dma_start(x1_t[:], x1_in[wrp])
            y1 = wk.tile([FEAT, N], dt.float32, tag="y1", bufs=1,
                         name=f"gy1_{wrp}")
            for hh in range(4):
                cps = pz.tile([FEAT, 3, BANKW], dt.float32, tag="f",
                              name=f"gc1_{wrp}_{hh}")
                for j in range(2):
                    mt = hh * 2 + j
                    nc.tensor.matmul(cps[:, j, 0:MT], w1dup_t[:],
                                     x1_t[:, mt * MT:(mt + 1) * MT],
                                     start=True, stop=True)
                dst = y1[:, hh * 2 * MT:(hh + 1) * 2 * MT] \
                    .rearrange("p (c w) -> p c w", w=MT)
                if eng == "act":
                    nc.scalar.activation(dst, cps[:, 0:2, 0:MT], Act.Relu,
                                         bias=b1dup_t[:])
                else:
                    nc.vector.tensor_scalar(dst, cps[:, 0:2, 0:MT], b1dup_t[:],
                                            0.0, Alu.add, Alu.max)
                yield
            is_a = wrp in (0, 2)
            gidx_src = gidxa_in[wrp // 2] if is_a else gidxb_in[wrp // 2]
            wbc_src = wbca_in[wrp // 2] if is_a else wbcb_in[wrp // 2]
            n_g = HALFG if is_a else N
            n_p = HALF if is_a else N
            zw = []
            for call in range(2):
                gidx_t = wk.tile([FEAT, n_g // 16], dt.int16, tag="gidx",
                                 name=f"ggi_{wrp}_{call}", bufs=2)
                nc.sync.dma_start(gidx_t[:], gidx_src[call])
                wbc_t = wk.tile([FEAT, n_p], dt.bfloat16, tag="wbc", bufs=1,
                                name=f"gwb_{wrp}_{call}")
                nc.sync.dma_start(wbc_t[:], wbc_src[call])
                z_t = wk.tile([FEAT, n_g], dt.float32, tag="z", bufs=1,
                              name=f"gz_{wrp}_{call}")
                nc.gpsimd.ap_gather(z_t[:], y1[:], gidx_t[:],
                                    channels=FEAT, num_elems=N, d=1,
                                    num_idxs=n_g)
                yield
                zw_t = wk.tile([FEAT, n_p], dt.bfloat16, tag=f"zw{call}",
                               bufs=1, name=f"gzw_{wrp}_{call}")
                zeng = nc.vector if zw_eng == "dve" else nc.gpsimd
                zeng.tensor_tensor(zw_t[:], z_t[:, 0:n_p], wbc_t[:], Alu.mult)
                zw.append(zw_t)
                yield
            f8tmp = wk.tile([FEAT, n_p], dt.float8e4, tag="f8tmp", bufs=1,
                            name=f"gf8t_{wrp}")
            for hh in range(n_p // (2 * MT)):
                cps2 = pz.tile([FEAT, 3, BANKW], dt.float32, tag="f",
                               name=f"gc2_{wrp}_{hh}")
                for j in range(2):
                    mt = hh * 2 + j
                    sl = slice(mt * MT, (mt + 1) * MT)
                    nc.tensor.matmul(cps2[:, j, 0:MT], w2r_t[:], zw[0][:, sl],
                                     start=True, stop=False)
                    nc.tensor.matmul(cps2[:, j, 0:MT], w2r_t[:], zw[1][:, sl],
                                     start=False, stop=True)
                dst = f8tmp[:, hh * 2 * MT:(hh + 1) * 2 * MT] \
                    .rearrange("p (c w) -> p c w", w=MT)
                if eng == "act":
                    nc.scalar.activation(dst, cps2[:, 0:2, 0:MT], Act.Identity,
                                         bias=b2_t[:])
                else:
                    nc.vector.tensor_scalar(dst, cps2[:, 0:2, 0:MT], b2_t[:],
                                            None, Alu.add)
                yield
            dstt = {0: f8ah, 2: f8av, 1: f8bh, 3: f8bv}[wrp]
            nc.sync.dma_start(dstt[:, 0, 0:n_p], f8tmp[0:64, :])
            nc.sync.dma_start(dstt[:, 1, 0:n_p], f8tmp[64:128, :])
            yield

        # ---------------- schedule --------------------------------------
        from contextlib import ExitStack as _ES
        import itertools

        fstate["oacc"] = pz.tile([NBLK, BANKW], dt.float32, tag="accA",
                                 name="oacc")
        fstate["uvacc"] = pz.tile([NBLK, BANKW], dt.float32, tag="accB",
                                  name="uvacc")

        wkstack = _ES()
        wk = wkstack.enter_context(tc.tile_pool(name="fwork", bufs=1))

        # warps 1 and 0 fully before phase R (ACT idle here)
        for _ in gen_warp(1, wk, None, "act"):
            pass
        for _ in gen_warp(0, wk, None, "act"):
            pass

        # warps 3 and 2 woven into phase R, relu/drain on DVE, zw on gpsimd
        wovens = itertools.chain(gen_warp(3, wk, None, "dve", zw_eng="gpsimd"),
                                 gen_warp(2, wk, None, "dve", zw_eng="gpsimd"))
        wov_active = True

        # F-chunk queue: sweep-major, then nbc-major within sweep
        fqueue = [(s, j, c) for s, js in enumerate(SWEEPJS)
                  for c in range(5) for j in js]
        f_next = 0
        f_open = False
        pending = []       # emitted fronts awaiting their matvecs
        g_done = -1
        ev_alt = itertools.cycle(["act", "dve"])
        drained = -1       # last sweep whose accumulators were drained

        def f_ready():
            if f_next >= len(fqueue):
                return False
            s, j, c = fqueue[f_next]
            if g_done < 3 * c + 2:
                return False
            if s > drained + 1:   # need previous sweep's accs drained
                return False
            return True

        def flush_pending(k=1):
            nonlocal_ns = pending[:k]
            del pending[:k]
            for fr in nonlocal_ns:
                emit_f_mvs(fr)

        def maybe_drain():
            # drain sweep s once all its chunks' matvecs are emitted
            nonlocal drained
            s = drained + 1
            n_done = sum(1 for i in range(f_next)
                         if fqueue[i][0] == s) - sum(1 for fr in pending
                                                    if JSWEEP[fr[0]] == s)
            if n_done == 5 * len(SWEEPJS[s]):
                emit_sweep_drain(s)
                drained = s

        JSWEEP = {}
        for _s, _js in enumerate(SWEEPJS):
            for _jv in _js:
                JSWEEP[_jv] = _s

        ftag_alt = itertools.cycle(["f", "r"])

        def f_slot(during_r):
            nonlocal f_next
            if pending:
                flush_pending(1)
                maybe_drain()
            if f_ready():
                s, j, c = fqueue[f_next]
                eng = "dve" if during_r else next(ev_alt)
                tag = "f" if during_r else next(ftag_alt)
                pending.append(emit_f_front(j, c, tag, eng))
                f_next += 1

        for nb in range(NNB):
            uhp = []
            for ci in range(3):
                uhp.append(emit_r_chunk(nb, ci))
                if wov_active:
                    for _ in range(2):
                        if next(wovens, "END") == "END":
                            wov_active = False
                            wkstack.close()
                            f_open = True
                            break
                elif f_open:
                    f_slot(during_r=True)
            emit_r_gfin(nb, uhp)
            g_done = nb

        while f_next < len(fqueue) or pending:
            f_slot(during_r=False)
        while drained < len(SWEEPJS) - 1:
            maybe_drain()

    nc.compile()
    return nc


def _get_nc(s):
    if s not in _CACHED:
        _CACHED[s] = _build(s)
    return _CACHED[s]


# ----------------------------------------------------------------------------
# Entry point
# ----------------------------------------------------------------------------

def kernel(**inputs):
    from concourse.bass_utils import run_bass_kernel_spmd

    in_maps = _host_prep(inputs)

    # One program for all 8 cores: the row-half each core handles is fully
    # encoded in its host-built gather tables.
    nc = _get_nc(0)
    last_err = None
    for attempt in range(3):
        try:
            r = run_bass_kernel_spmd(nc, in_maps, core_ids=list(range(NCORES)))
            break
        except Exception as e:  # transient NRT_EXEC_UNIT_UNRECOVERABLE wedges
            last_err = e
            import time
            time.sleep(10 * (attempt + 1))
    else:
        raise last_err
    results = r.results

    # host combine (exact)
    def _gather_m(arr):
        out = np.zeros(N, np.float64)
        for s, js in enumerate(SWEEPJS):
            for p, j in enumerate(js):
                out[j * MT:(j + 1) * MT] = arr[s, p].astype(np.float64)
        return out

    logs = np.zeros((B, N), np.float64)
    for b in range(B):
        r0, r1 = results[2 * b], results[2 * b + 1]
        O = _gather_m(r0["o_out"]) + _gather_m(r1["o_out"])
        uv = _gather_m(r0["uv_out"]) + _gather_m(r1["uv_out"])
        res_sum = O / uv
        logs[b] = np.log(res_sum + 1e-4)
    return np.float32(logs.mean())


# revision 32
# speedup vs baseline: 1.9279x; 1.8526x over previous
"""Trainium2 Bass kernel for nn_FMAPModelWarping (retrieval_knn).

The host does the cheap per-pixel prep (affine grids, bilinear taps, the
3x3x3->64 and 1x1 convs, 4-tap backward warp — ~1 GFLOP total, <4% of the
model) and ships fp8 feature maps. The 8 NeuronCores do the FLOP-heavy
part (~26.5 GFLOP): two 3600x3600x128 correlations per sample and the
bidirectional-softmax reduction, tiled flash-attention-style.

Sharding: core k = 2*b + s handles sample b (of 4) and row-half s of the
3600x3600 correlation matrices; partial column stats combine on the host.

Math restructure (exact):
  g[n] = 1/U_h[n],  res_sum[m] = O[m] / U_v[m],
  O[m] = sum_n g[n] * eh[n,m] * ev[n,m]
with U_h = rowsum(eh), U_v = colsum(ev), eh = exp(Mh), ev = exp(Mv).

Device structure: features live in a dual-plane [64, 2, n] fp8 layout so
the correlation matmuls run in DoubleRow perf mode (256-deep contraction,
0.5 cycles/row). Phase R computes exp(Mh) row-blocks (kept in SBUF) with
the row sums coming free from the activation engine's accumulator; phase F
(m-outer) recomputes exp(Mv), forms t = eh*ev, and accumulates O and U_v
via PSUM matvecs. R-chunks and F-chunks share a two-deep PSUM rotation and
are interleaved so the ACT exp stream, DVE/Pool elementwise work and PE
matmuls all overlap; part of the exp(Mv) field uses a Schraudolph bit-trick
exponential on DVE (its small relative noise cancels between O and U_v,
which consume the same ev values).
"""

import numpy as np

B, C_IN, H, W = 4, 3, 60, 60
HID, FEAT = 64, 128
N = H * W               # 3600
NCORES = 8
HALF = N // 2           # 1800 rows per core
NBLK = 128              # correlation row-block (partition dim)
NNB = 15                # row blocks per core (15*128 = 1920, rows padded)
NPAD = NNB * NBLK       # 1920
MT = 450                # m-tile width
N_MT = N // MT          # 8 m tiles
BANKW = 512             # fp32 elems per PSUM bank

# Schraudolph constants (bf16 target): i16 = rne(x*SA + SB); bits as bf16.
SA = 128.0 / float(np.log(2.0))
SB = 127.0 * 128.0 - 5.5 - 1.86   # -1.86 centers the measured +1% bias

# m-tile groups sharing the PSUM accumulator banks (4 rows at partition
# offsets 0/32/64/96 per bank; O and U_v each get one bank).
SWEEPJS = [(0, 1, 2, 3), (4, 5, 6, 7)]


# ----------------------------------------------------------------------------
# Host-side prep: exact reference semantics for grids / bilinear taps / rolls
# ----------------------------------------------------------------------------

def _affine_coords(theta2x3):
    """Pixel-space sample coords (x, y) for torch affine_grid+grid_sample
    (align_corners=False), shape [H, W] each."""
    xs = (2.0 * np.arange(W, dtype=np.float64) + 1.0) / W - 1.0
    ys = (2.0 * np.arange(H, dtype=np.float64) + 1.0) / H - 1.0
    gx, gy = np.meshgrid(xs, ys)           # gx[i,j]=xs[j], gy[i,j]=ys[i]
    t = theta2x3.astype(np.float64)
    cx = t[0, 0] * gx + t[0, 1] * gy + t[0, 2]
    cy = t[1, 0] * gx + t[1, 1] * gy + t[1, 2]
    px = (cx + 1.0) * W * 0.5 - 0.5
    py = (cy + 1.0) * H * 0.5 - 0.5
    return px, py


def _bilinear_sample_host(img, px, py):
    """img [C,H,W] float32, sample at (px,py) [H,W]; zeros padding.
    Mirrors reference grid_sample exactly."""
    x0 = np.floor(px); y0 = np.floor(py)
    wx1 = (px - x0); wx0 = 1.0 - wx1
    wy1 = (py - y0); wy0 = 1.0 - wy1
    out = np.zeros((img.shape[0],) + px.shape, np.float64)
    flat = img.reshape(img.shape[0], -1).astype(np.float64)
    for ix, iy, wt in ((x0, y0, wx0 * wy0), (x0 + 1, y0, wx1 * wy0),
                       (x0, y0 + 1, wx0 * wy1), (x0 + 1, y0 + 1, wx1 * wy1)):
        valid = (ix >= 0) & (ix < W) & (iy >= 0) & (iy < H)
        ii = np.clip(ix, 0, W - 1).astype(np.int64)
        jj = np.clip(iy, 0, H - 1).astype(np.int64)
        v = flat[:, (jj * W + ii).ravel()].reshape(out.shape)
        out += v * (wt * valid)[None]
    return out.astype(np.float32)


def _back_taps(theta2x3, u, v):
    """Tap indices/weights for grid_sample(y, grid(Bm)) composed with the
    inverse roll. Returns idx [4,3600] int (in-range), wt [4,3600] f32."""
    px, py = _affine_coords(theta2x3)
    ii = np.arange(H)[:, None]; jj = np.arange(W)[None, :]
    qi = (ii - u) % H; qj = (jj - v) % W
    xs = px[qi, qj].ravel(); ys = py[qi, qj].ravel()
    x0 = np.floor(xs); y0 = np.floor(ys)
    fx = xs - x0; fy = ys - y0
    idxs, wts = [], []
    for ix, iy, wt in ((x0, y0, (1 - fx) * (1 - fy)), (x0 + 1, y0, fx * (1 - fy)),
                       (x0, y0 + 1, (1 - fx) * fy), (x0 + 1, y0 + 1, fx * fy)):
        valid = (ix >= 0) & (ix < W) & (iy >= 0) & (iy < H)
        cii = np.clip(ix, 0, W - 1).astype(np.int64)
        cjj = np.clip(iy, 0, H - 1).astype(np.int64)
        idxs.append(cjj * W + cii)
        wts.append((wt * valid).astype(np.float32))
    return np.stack(idxs), np.stack(wts)


def _host_prep(inputs):
    """Build the 8 per-core device input dicts (fp8 dual-plane features)."""
    import ml_dtypes
    x_a = np.asarray(inputs["input_a"], np.float32)
    x_b = np.asarray(inputs["input_b"], np.float32)
    w1 = np.asarray(inputs["w1"], np.float32)
    b1 = np.asarray(inputs["b1"], np.float32)
    w2 = np.asarray(inputs["w2"], np.float32)
    b2 = np.asarray(inputs["b2"], np.float32)
    noise = np.asarray(inputs["noise"], np.float32)
    u_roll = np.asarray(inputs["u_roll"])
    v_roll = np.asarray(inputs["v_roll"])
    swap = np.asarray(inputs["swap"])

    w1mat = w1.reshape(HID, C_IN * 9)                  # [64, 27]
    w2mat = w2.reshape(FEAT, HID)                      # [128, 64]

    eye = np.eye(3, dtype=np.float64)
    mask = np.array([[1., 1., 1.], [1., 1., 1.], [0., 0., 0.]])

    # F[wrp][b]: warped feature map [FEAT, N] float32 (exact reference math;
    # the 1x1 conv2 commutes with the backward spatial gather)
    F = np.zeros((4, B, FEAT, N), np.float32)
    for wrp in range(4):
        sw = int(swap[wrp]) == 1
        for b in range(B):
            fwd = eye + 0.05 * noise[wrp, b].astype(np.float64) * mask
            bwd = np.linalg.inv(fwd)
            A_ = bwd if sw else fwd
            Bm = fwd if sw else bwd
            u = int(u_roll[wrp, b]); v = int(v_roll[wrp, b])
            img = x_a[b] if wrp in (0, 2) else x_b[b]
            x_r = np.roll(np.roll(img, -u, axis=1), -v, axis=2)
            px, py = _affine_coords(np.asarray(A_)[:2])
            xw = _bilinear_sample_host(x_r, px, py)       # [3,60,60]
            # im2col, zero-pad SAME, k = c*9 + ky*3 + kx
            pad = np.zeros((C_IN, H + 2, W + 2), np.float32)
            pad[:, 1:-1, 1:-1] = xw
            X1 = np.zeros((C_IN * 9, N), np.float32)
            k = 0
            for c in range(C_IN):
                for ky in range(3):
                    for kx in range(3):
                        X1[k] = pad[c, ky:ky + H, kx:kx + W].ravel()
                        k += 1
            y1 = np.maximum(w1mat @ X1 + b1[:, None], 0.0)   # [64, N]
            y2 = w2mat @ y1 + b2[:, None]                    # [128, N]
            idx, wt = _back_taps(np.asarray(Bm)[:2], u, v)
            Fw = np.zeros((FEAT, N), np.float32)
            for tap in range(4):
                Fw += y2[:, idx[tap]] * wt[tap][None, :]
            F[wrp, b] = Fw

    F8 = F.astype(ml_dtypes.float8_e4m3fn)

    def dual_plane(feat, cols):
        """[FEAT, n] -> [64, 2, cols] (zero-padded)."""
        out = np.zeros((64, 2, cols), ml_dtypes.float8_e4m3fn)
        n = feat.shape[1]
        out[:, 0, :n] = feat[0:64]
        out[:, 1, :n] = feat[64:128]
        return out

    # U_v matvec stationaries: ones, except block 14 masks the 120 pad rows
    onesmask = np.ones((128, NNB), np.float32)
    onesmask[8:, NNB - 1] = 0.0

    in_maps = []
    for core in range(NCORES):
        b = core // 2
        s = core % 2
        n0 = s * HALF
        in_maps.append({
            "f8ah_in": dual_plane(F8[0, b][:, n0:n0 + HALF], NPAD),
            "f8av_in": dual_plane(F8[2, b][:, n0:n0 + HALF], NPAD),
            "f8bh_in": dual_plane(F8[1, b], N),
            "f8bv_in": dual_plane(F8[3, b], N),
            "onesmask_in": onesmask.astype(ml_dtypes.bfloat16),
        })
    return in_maps


# ----------------------------------------------------------------------------
# Device kernel builder
# ----------------------------------------------------------------------------

_CACHED = {}


def _build(core_half):
    """Build the Bacc module (one NEFF shared by all 8 cores; each core's
    sample/row-half is fully encoded in its host-built feature tiles)."""
    import concourse.bacc as bacc_mod
    import concourse.mybir as mybir
    from concourse.tile import TileContext
    from contextlib import ExitStack
    import itertools

    dt = mybir.dt
    Alu = mybir.AluOpType
    Act = mybir.ActivationFunctionType
    DR = mybir.MatmulPerfMode.DoubleRow

    nc = bacc_mod.Bacc("TRN2", target_bir_lowering=False)

    f8ah_in = nc.dram_tensor("f8ah_in", [64, 2, NPAD], dt.float8e4, kind="ExternalInput")
    f8av_in = nc.dram_tensor("f8av_in", [64, 2, NPAD], dt.float8e4, kind="ExternalInput")
    f8bh_in = nc.dram_tensor("f8bh_in", [64, 2, N], dt.float8e4, kind="ExternalInput")
    f8bv_in = nc.dram_tensor("f8bv_in", [64, 2, N], dt.float8e4, kind="ExternalInput")
    onesmask_in = nc.dram_tensor("onesmask_in", [FEAT, NNB], dt.bfloat16, kind="ExternalInput")

    o_out = nc.dram_tensor("o_out", [2, 4, MT], dt.float32, kind="ExternalOutput")
    uv_out = nc.dram_tensor("uv_out", [2, 4, MT], dt.float32, kind="ExternalOutput")

    with ExitStack() as ctx:
        tc = ctx.enter_context(TileContext(nc))

        const = ctx.enter_context(tc.tile_pool(name="const", bufs=1))
        onesmask_t = const.tile([FEAT, NNB], dt.bfloat16)
        nc.sync.dma_start(onesmask_t[:], onesmask_in[:])

        fpool = ctx.enter_context(tc.tile_pool(name="feat", bufs=1))
        f8ah = fpool.tile([64, 2, NPAD], dt.float8e4, name="f8ah")
        f8av = fpool.tile([64, 2, NPAD], dt.float8e4, name="f8av")
        f8bh = fpool.tile([64, 2, N], dt.float8e4, name="f8bh")
        f8bv = fpool.tile([64, 2, N], dt.float8e4, name="f8bv")
        nc.sync.dma_start(f8ah[:], f8ah_in[:])
        nc.sync.dma_start(f8bh[:], f8bh_in[:])
        nc.sync.dma_start(f8av[:], f8av_in[:])
        nc.sync.dma_start(f8bv[:], f8bv_in[:])

        stat = ctx.enter_context(tc.tile_pool(name="stat", bufs=1))
        eh_t = [stat.tile([NBLK, N], dt.bfloat16, name=f"eh{_nb}")
                for _nb in range(NNB)]
        g_t = [stat.tile([NBLK, 1], dt.bfloat16, name=f"g{_nb}")
               for _nb in range(NNB)]

        rs = ctx.enter_context(tc.tile_pool(name="rsmall", bufs=10))

        # Single PSUM pool, bank budget 8: tag "u" (2 x 3 banks, a unified
        # two-deep rotation shared by R chunks and F chunks — consecutive
        # chunks land in alternating buffers so each chunk's matmuls overlap
        # the previous chunk's exp), plus two accumulator banks (4 rows each
        # at partition offsets 0/32/64/96).
        pz = ctx.enter_context(tc.tile_pool(name="pz", bufs=1, space="PSUM"))
        fwkpool = ctx.enter_context(tc.tile_pool(name="fwkpool", bufs=1))

        oacc = pz.tile([NBLK, BANKW], dt.float32, tag="accA", name="oacc")
        uvacc = pz.tile([NBLK, BANKW], dt.float32, tag="accB", name="uvacc")

        # R chunking: m ranges as (start, n_tiles) with 450-wide tiles
        RCH = [(0, 3), (1350, 3), (2700, 2)]

        def emit_r_chunk(nb, ci):
            nsl = slice(nb * NBLK, (nb + 1) * NBLK)
            m0, nt = RCH[ci]
            rt = pz.tile([NBLK, 3, BANKW], dt.float32, tag="u", bufs=2,
                         name=f"r_{nb}_{ci}")
            for k in range(nt):
                nc.tensor.matmul(rt[:, k, 0:MT], f8ah[:, :, nsl],
                                 f8bh[:, :, m0 + k * MT: m0 + (k + 1) * MT],
                                 start=True, stop=True, perf_mode=DR)
            ehv = eh_t[nb][:, m0: m0 + nt * MT] \
                .rearrange("p (c w) -> p c w", w=MT)
            uh = rs.tile([NBLK, 1], dt.float32, tag=f"uh{ci}",
                         name=f"uh_{nb}_{ci}")
            nc.scalar.activation(ehv, rt[:, 0:nt, 0:MT], Act.Exp,
                                 accum_out=uh[:])
            return uh

        def emit_r_gfin(nb, uhp):
            ua = rs.tile([NBLK, 1], dt.float32, tag="ua", name=f"ua_{nb}")
            nc.vector.tensor_tensor(ua[:], uhp[0][:], uhp[1][:], Alu.add)
            nc.vector.tensor_tensor(ua[:], ua[:], uhp[2][:], Alu.add)
            gr = rs.tile([NBLK, 1], dt.float32, tag="gr", name=f"gr_{nb}")
            nc.vector.reciprocal(gr[:], ua[:])
            nc.vector.tensor_copy(g_t[nb][:], gr[:])
            if nb == NNB - 1:   # zero g on the 120 pad rows
                nc.vector.tensor_tensor(g_t[nb][:], g_t[nb][:],
                                        onesmask_t[:, nb:nb + 1], Alu.mult)

        # ---------------- phase F chunk ---------------------------------
        JPOS = {}
        JSWEEP = {}
        for _s, _js in enumerate(SWEEPJS):
            for _p, _jv in enumerate(_js):
                JPOS[_jv] = _p
                JSWEEP[_jv] = _s

        def emit_f_front(j, nbc, ev_eng, t_eng):
            jsl = slice(j * MT, (j + 1) * MT)
            nbs = [3 * nbc + k for k in range(3)]
            ft = pz.tile([NBLK, 3, BANKW], dt.float32, tag="u", bufs=2,
                         name=f"f_{j}_{nbc}")
            for kk, nb in enumerate(nbs):
                nsl = slice(nb * NBLK, (nb + 1) * NBLK)
                nc.tensor.matmul(ft[:, kk, 0:MT], f8av[:, :, nsl],
                                 f8bv[:, :, jsl], start=True, stop=True,
                                 perf_mode=DR)
            ev_i = fwkpool.tile([NBLK, 3, MT], dt.int16, tag="ev", bufs=4,
                                name=f"ev_{j}_{nbc}")
            evb = ev_i[:].bitcast(dt.bfloat16)
            if ev_eng == "act":
                nc.scalar.activation(evb, ft[:, :, 0:MT], Act.Exp)
            else:
                nc.vector.tensor_scalar(ev_i[:], ft[:, :, 0:MT], SA, SB,
                                        Alu.mult, Alu.add)
            t_t = fwkpool.tile([NBLK, 3, MT], dt.bfloat16, tag="t", bufs=4,
                               name=f"t_{j}_{nbc}")
            teng = nc.vector if t_eng == "dve" else nc.gpsimd
            for kk, nb in enumerate(nbs):
                teng.tensor_tensor(t_t[:, kk, :], eh_t[nb][:, jsl],
                                   ev_i[:, kk, :].bitcast(dt.bfloat16),
                                   Alu.mult)
            return (j, nbc, t_t, ev_i)

        def emit_f_mvs(front):
            j, nbc, t_t, ev_i = front
            jj = JPOS[j]
            nbs = [3 * nbc + k for k in range(3)]
            orow = oacc[32 * jj:32 * jj + 1, 0:MT]
            uvrow = uvacc[32 * jj:32 * jj + 1, 0:MT]
            for kk, nb in enumerate(nbs):
                nc.tensor.matmul(orow, g_t[nb][:], t_t[:, kk, :],
                                 start=(nb == 0), stop=(nb == NNB - 1),
                                 skip_group_check=True,
                                 tile_position=(0, 32 * jj))
                nc.tensor.matmul(uvrow, onesmask_t[:, nb:nb + 1],
                                 ev_i[:, kk, :].bitcast(dt.bfloat16),
                                 start=(nb == 0), stop=(nb == NNB - 1),
                                 skip_group_check=True,
                                 tile_position=(0, 32 * jj))

        def emit_sweep_drain(s):
            osb = fwkpool.tile([128, MT], dt.float32, tag="osb", bufs=1,
                               name=f"osb_{s}")
            uvsb = fwkpool.tile([128, MT], dt.float32, tag="uvsb", bufs=1,
                                name=f"uvsb_{s}")
            nc.scalar.copy(osb[:], oacc[:, 0:MT])
            nc.scalar.copy(uvsb[:], uvacc[:, 0:MT])
            ov = osb[:].rearrange("(q t) m -> q t m", t=32)[:, 0, :]
            uvv = uvsb[:].rearrange("(q t) m -> q t m", t=32)[:, 0, :]
            nc.sync.dma_start(o_out[s], ov)
            nc.sync.dma_start(uv_out[s], uvv)

        # ---------------- schedule --------------------------------------
        # F-chunk queue: sweep-major, then nbc-major within sweep
        fqueue = [(s, j, c) for s, js in enumerate(SWEEPJS)
                  for c in range(5) for j in js]
        f_next = 0
        pending = []       # emitted fronts awaiting their matvecs
        g_done = -1
        drained = -1       # last sweep whose accumulators were drained
        ev_alt = itertools.cycle(["act", "act", "act", "act", "dve"])
        t_alt = itertools.cycle(["dve", "dve", "pool"])

        def f_ready():
            if f_next >= len(fqueue):
                return False
            s, j, c = fqueue[f_next]
            if g_done < 3 * c + 2:
                return False
            if s > drained + 1:   # need previous sweep's accs drained
                return False
            return True

        def maybe_drain():
            nonlocal drained
            s = drained + 1
            if s >= len(SWEEPJS):
                return
            n_done = sum(1 for i in range(f_next)
                         if fqueue[i][0] == s) - sum(1 for fr in pending
                                                    if JSWEEP[fr[0]] == s)
            if n_done == 5 * len(SWEEPJS[s]):
                emit_sweep_drain(s)
                drained = s

        def f_slot(during_r):
            nonlocal f_next
            lag = 2
            if (len(pending) >= lag + (1 if f_ready() else 0)) or \
                    (pending and not f_ready()):
                emit_f_mvs(pending.pop(0))
                maybe_drain()
            if f_ready():
                s, j, c = fqueue[f_next]
                eng = "dve" if during_r else next(ev_alt)
                pending.append(emit_f_front(j, c, eng, next(t_alt)))
                f_next += 1

        import os
        _phase = os.environ.get("KPHASE", "all")
        for nb in range(NNB):
            uhp = []
            for ci in range(3):
                uhp.append(emit_r_chunk(nb, ci))
                f_slot(during_r=True)
            emit_r_gfin(nb, uhp)
            g_done = nb
        if _phase == "all":
            while f_next < len(fqueue) or pending:
                f_slot(during_r=False)
            while drained < len(SWEEPJS) - 1:
                maybe_drain()

    nc.compile()
    return nc


def _get_nc(s):
    if s not in _CACHED:
        _CACHED[s] = _build(s)
    return _CACHED[s]


# ----------------------------------------------------------------------------
# Entry point
# ----------------------------------------------------------------------------

def kernel(**inputs):
    from concourse.bass_utils import run_bass_kernel_spmd

    in_maps = _host_prep(inputs)

    # One program for all 8 cores: the sample/row-half each core handles is
    # fully encoded in its host-built feature tiles.
    nc = _get_nc(0)
    last_err = None
    for attempt in range(3):
        try:
            r = run_bass_kernel_spmd(nc, in_maps, core_ids=list(range(NCORES)))
            break
        except Exception as e:  # transient NRT_EXEC_UNIT_UNRECOVERABLE wedges
            last_err = e
            import time
            time.sleep(10 * (attempt + 1))
    else:
        raise last_err
    results = r.results

    # host combine (exact)
    def _gather_m(arr):
        out = np.zeros(N, np.float64)
        for s, js in enumerate(SWEEPJS):
            for p, j in enumerate(js):
                out[j * MT:(j + 1) * MT] = arr[s, p].astype(np.float64)
        return out

    logs = np.zeros((B, N), np.float64)
    for b in range(B):
        r0, r1 = results[2 * b], results[2 * b + 1]
        O = _gather_m(r0["o_out"]) + _gather_m(r1["o_out"])
        uv = _gather_m(r0["uv_out"]) + _gather_m(r1["uv_out"])
        res_sum = O / uv
        logs[b] = np.log(res_sum + 1e-4)
    return np.float32(logs.mean())


# revision 44
# speedup vs baseline: 2.1428x; 1.1115x over previous
"""Trainium2 Bass kernel for nn_FMAPModelWarping (retrieval_knn).

The host does the cheap per-pixel prep (affine grids, bilinear taps, the
3x3x3->64 and 1x1 convs, 4-tap backward warp — ~1 GFLOP total, <4% of the
model) and ships fp8 feature maps. The 8 NeuronCores do the FLOP-heavy
part (~26.5 GFLOP): two 3600x3600x128 correlations per sample and the
bidirectional-softmax reduction, tiled flash-attention-style.

Sharding: core k = 2*b + s handles sample b (of 4) and row-half s of the
3600x3600 correlation matrices; partial column stats combine on the host.

Math restructure (exact):
  g[n] = 1/U_h[n],  res_sum[m] = O[m] / U_v[m],
  O[m] = sum_n g[n] * eh[n,m] * ev[n,m]
with U_h = rowsum(eh), U_v = colsum(ev), eh = exp(Mh), ev = exp(Mv).

Device structure: features live in a dual-plane [64, 2, n] fp8 layout so
the correlation matmuls run in DoubleRow perf mode (256-deep contraction,
0.5 cycles/row). Phase R computes exp(Mh) row-blocks (kept in SBUF) with
the row sums coming free from the activation engine's accumulator; phase F
(m-outer) recomputes exp(Mv), forms t = eh*ev, and accumulates O and U_v
via PSUM matvecs. R-chunks and F-chunks share a two-deep PSUM rotation and
are interleaved so the ACT exp stream, DVE/Pool elementwise work and PE
matmuls all overlap; part of the exp(Mv) field uses a Schraudolph bit-trick
exponential on DVE (its small relative noise cancels between O and U_v,
which consume the same ev values).
"""

import numpy as np

B, C_IN, H, W = 4, 3, 60, 60
HID, FEAT = 64, 128
N = H * W               # 3600
NCORES = 8
HALF = N // 2           # 1800 rows per core
NBLK = 128              # correlation row-block (partition dim)
NNB = 15                # row blocks per core (15*128 = 1920, rows padded)
NPAD = NNB * NBLK       # 1920
MT = 450                # m-tile width
N_MT = N // MT          # 8 m tiles
BANKW = 512             # fp32 elems per PSUM bank

# Schraudolph constants (bf16 target): i16 = rne(x*SA + SB); bits as bf16.
SA = 128.0 / float(np.log(2.0))
SB = 127.0 * 128.0 - 5.5 - 1.86   # -1.86 centers the measured +1% bias

# m-tile groups sharing the PSUM accumulator banks (4 rows at partition
# offsets 0/32/64/96 per bank; O and U_v each get one bank).
SWEEPJS = [(0, 1, 2, 3), (4, 5, 6, 7)]


# ----------------------------------------------------------------------------
# Host-side prep: exact reference semantics for grids / bilinear taps / rolls
# ----------------------------------------------------------------------------

def _affine_coords(theta2x3):
    """Pixel-space sample coords (x, y) for torch affine_grid+grid_sample
    (align_corners=False), shape [H, W] each."""
    xs = (2.0 * np.arange(W, dtype=np.float64) + 1.0) / W - 1.0
    ys = (2.0 * np.arange(H, dtype=np.float64) + 1.0) / H - 1.0
    gx, gy = np.meshgrid(xs, ys)           # gx[i,j]=xs[j], gy[i,j]=ys[i]
    t = theta2x3.astype(np.float64)
    cx = t[0, 0] * gx + t[0, 1] * gy + t[0, 2]
    cy = t[1, 0] * gx + t[1, 1] * gy + t[1, 2]
    px = (cx + 1.0) * W * 0.5 - 0.5
    py = (cy + 1.0) * H * 0.5 - 0.5
    return px, py


def _bilinear_sample_host(img, px, py):
    """img [C,H,W] float32, sample at (px,py) [H,W]; zeros padding.
    Mirrors reference grid_sample exactly."""
    x0 = np.floor(px); y0 = np.floor(py)
    wx1 = (px - x0); wx0 = 1.0 - wx1
    wy1 = (py - y0); wy0 = 1.0 - wy1
    out = np.zeros((img.shape[0],) + px.shape, np.float64)
    flat = img.reshape(img.shape[0], -1).astype(np.float64)
    for ix, iy, wt in ((x0, y0, wx0 * wy0), (x0 + 1, y0, wx1 * wy0),
                       (x0, y0 + 1, wx0 * wy1), (x0 + 1, y0 + 1, wx1 * wy1)):
        valid = (ix >= 0) & (ix < W) & (iy >= 0) & (iy < H)
        ii = np.clip(ix, 0, W - 1).astype(np.int64)
        jj = np.clip(iy, 0, H - 1).astype(np.int64)
        v = flat[:, (jj * W + ii).ravel()].reshape(out.shape)
        out += v * (wt * valid)[None]
    return out.astype(np.float32)


def _back_taps(theta2x3, u, v):
    """Tap indices/weights for grid_sample(y, grid(Bm)) composed with the
    inverse roll. Returns idx [4,3600] int (in-range), wt [4,3600] f32."""
    px, py = _affine_coords(theta2x3)
    ii = np.arange(H)[:, None]; jj = np.arange(W)[None, :]
    qi = (ii - u) % H; qj = (jj - v) % W
    xs = px[qi, qj].ravel(); ys = py[qi, qj].ravel()
    x0 = np.floor(xs); y0 = np.floor(ys)
    fx = xs - x0; fy = ys - y0
    idxs, wts = [], []
    for ix, iy, wt in ((x0, y0, (1 - fx) * (1 - fy)), (x0 + 1, y0, fx * (1 - fy)),
                       (x0, y0 + 1, (1 - fx) * fy), (x0 + 1, y0 + 1, fx * fy)):
        valid = (ix >= 0) & (ix < W) & (iy >= 0) & (iy < H)
        cii = np.clip(ix, 0, W - 1).astype(np.int64)
        cjj = np.clip(iy, 0, H - 1).astype(np.int64)
        idxs.append(cjj * W + cii)
        wts.append((wt * valid).astype(np.float32))
    return np.stack(idxs), np.stack(wts)


def _host_prep(inputs):
    """Build the 8 per-core device input dicts (fp8 dual-plane features)."""
    import ml_dtypes
    x_a = np.asarray(inputs["input_a"], np.float32)
    x_b = np.asarray(inputs["input_b"], np.float32)
    w1 = np.asarray(inputs["w1"], np.float32)
    b1 = np.asarray(inputs["b1"], np.float32)
    w2 = np.asarray(inputs["w2"], np.float32)
    b2 = np.asarray(inputs["b2"], np.float32)
    noise = np.asarray(inputs["noise"], np.float32)
    u_roll = np.asarray(inputs["u_roll"])
    v_roll = np.asarray(inputs["v_roll"])
    swap = np.asarray(inputs["swap"])

    w1mat = w1.reshape(HID, C_IN * 9)                  # [64, 27]
    w2mat = w2.reshape(FEAT, HID)                      # [128, 64]

    eye = np.eye(3, dtype=np.float64)
    mask = np.array([[1., 1., 1.], [1., 1., 1.], [0., 0., 0.]])

    # F[wrp][b]: warped feature map [FEAT, N] float32 (exact reference math;
    # the 1x1 conv2 commutes with the backward spatial gather)
    F = np.zeros((4, B, FEAT, N), np.float32)
    for wrp in range(4):
        sw = int(swap[wrp]) == 1
        for b in range(B):
            fwd = eye + 0.05 * noise[wrp, b].astype(np.float64) * mask
            bwd = np.linalg.inv(fwd)
            A_ = bwd if sw else fwd
            Bm = fwd if sw else bwd
            u = int(u_roll[wrp, b]); v = int(v_roll[wrp, b])
            img = x_a[b] if wrp in (0, 2) else x_b[b]
            x_r = np.roll(np.roll(img, -u, axis=1), -v, axis=2)
            px, py = _affine_coords(np.asarray(A_)[:2])
            xw = _bilinear_sample_host(x_r, px, py)       # [3,60,60]
            # im2col, zero-pad SAME, k = c*9 + ky*3 + kx
            pad = np.zeros((C_IN, H + 2, W + 2), np.float32)
            pad[:, 1:-1, 1:-1] = xw
            X1 = np.zeros((C_IN * 9, N), np.float32)
            k = 0
            for c in range(C_IN):
                for ky in range(3):
                    for kx in range(3):
                        X1[k] = pad[c, ky:ky + H, kx:kx + W].ravel()
                        k += 1
            y1 = np.maximum(w1mat @ X1 + b1[:, None], 0.0)   # [64, N]
            y2 = w2mat @ y1 + b2[:, None]                    # [128, N]
            idx, wt = _back_taps(np.asarray(Bm)[:2], u, v)
            Fw = np.zeros((FEAT, N), np.float32)
            for tap in range(4):
                Fw += y2[:, idx[tap]] * wt[tap][None, :]
            F[wrp, b] = Fw

    F8 = F.astype(ml_dtypes.float8_e4m3fn)

    def dual_plane(feat, cols):
        """[FEAT, n] -> [64, 2, cols] (zero-padded)."""
        out = np.zeros((64, 2, cols), ml_dtypes.float8_e4m3fn)
        n = feat.shape[1]
        out[:, 0, :n] = feat[0:64]
        out[:, 1, :n] = feat[64:128]
        return out

    # U_v matvec stationaries: ones, except block 14 masks the 120 pad rows
    onesmask = np.ones((128, NNB), np.float32)
    onesmask[8:, NNB - 1] = 0.0

    in_maps = []
    for core in range(NCORES):
        b = core // 2
        s = core % 2
        n0 = s * HALF
        in_maps.append({
            "f8ah_in": dual_plane(F8[0, b][:, n0:n0 + HALF], NPAD),
            "f8av_in": dual_plane(F8[2, b][:, n0:n0 + HALF], NPAD),
            "f8bh_in": dual_plane(F8[1, b], N),
            "f8bv_in": dual_plane(F8[3, b], N),
            "onesmask_in": onesmask.astype(ml_dtypes.bfloat16),
        })
    return in_maps


# ----------------------------------------------------------------------------
# Device kernel builder
# ----------------------------------------------------------------------------

_CACHED = {}


def _build(core_half):
    """Build the Bacc module (one NEFF shared by all 8 cores; each core's
    sample/row-half is fully encoded in its host-built feature tiles)."""
    import concourse.bacc as bacc_mod
    import concourse.mybir as mybir
    from concourse.tile import TileContext
    from contextlib import ExitStack
    import itertools

    dt = mybir.dt
    Alu = mybir.AluOpType
    Act = mybir.ActivationFunctionType
    DR = mybir.MatmulPerfMode.DoubleRow

    nc = bacc_mod.Bacc("TRN2", target_bir_lowering=False)

    f8ah_in = nc.dram_tensor("f8ah_in", [64, 2, NPAD], dt.float8e4, kind="ExternalInput")
    f8av_in = nc.dram_tensor("f8av_in", [64, 2, NPAD], dt.float8e4, kind="ExternalInput")
    f8bh_in = nc.dram_tensor("f8bh_in", [64, 2, N], dt.float8e4, kind="ExternalInput")
    f8bv_in = nc.dram_tensor("f8bv_in", [64, 2, N], dt.float8e4, kind="ExternalInput")
    onesmask_in = nc.dram_tensor("onesmask_in", [FEAT, NNB], dt.bfloat16, kind="ExternalInput")

    o_out = nc.dram_tensor("o_out", [2, 4, MT], dt.float32, kind="ExternalOutput")
    uv_out = nc.dram_tensor("uv_out", [2, 4, MT], dt.float32, kind="ExternalOutput")

    with ExitStack() as ctx:
        tc = ctx.enter_context(TileContext(nc))

        const = ctx.enter_context(tc.tile_pool(name="const", bufs=1))
        onesmask_t = const.tile([FEAT, NNB], dt.bfloat16)
        nc.sync.dma_start(onesmask_t[:], onesmask_in[:])

        fpool = ctx.enter_context(tc.tile_pool(name="feat", bufs=1))
        f8ah = fpool.tile([64, 2, NPAD], dt.float8e4, name="f8ah")
        f8av = fpool.tile([64, 2, NPAD], dt.float8e4, name="f8av")
        f8bh = fpool.tile([64, 2, N], dt.float8e4, name="f8bh")
        f8bv = fpool.tile([64, 2, N], dt.float8e4, name="f8bv")
        nc.sync.dma_start(f8ah[:], f8ah_in[:])
        nc.sync.dma_start(f8bh[:], f8bh_in[:])
        nc.sync.dma_start(f8av[:], f8av_in[:])
        nc.sync.dma_start(f8bv[:], f8bv_in[:])

        stat = ctx.enter_context(tc.tile_pool(name="stat", bufs=1))
        eh_t = [stat.tile([NBLK, N], dt.bfloat16, name=f"eh{_nb}")
                for _nb in range(NNB)]
        g_t = [stat.tile([NBLK, 1], dt.bfloat16, name=f"g{_nb}")
               for _nb in range(NNB)]

        rs = ctx.enter_context(tc.tile_pool(name="rsmall", bufs=10))

        # Single PSUM pool, bank budget 8: tag "u" (2 x 3 banks, a unified
        # two-deep rotation shared by R chunks and F chunks — consecutive
        # chunks land in alternating buffers so each chunk's matmuls overlap
        # the previous chunk's exp), plus two accumulator banks (4 rows each
        # at partition offsets 0/32/64/96).
        pz = ctx.enter_context(tc.tile_pool(name="pz", bufs=1, space="PSUM"))
        fwkpool = ctx.enter_context(tc.tile_pool(name="fwkpool", bufs=1))

        oacc = pz.tile([NBLK, BANKW], dt.float32, tag="accA", name="oacc")
        uvacc = pz.tile([NBLK, BANKW], dt.float32, tag="accB", name="uvacc")

        # R chunking: m ranges as (start, n_tiles) with 450-wide tiles
        RCH = [(0, 3), (1350, 3), (2700, 2)]

        def emit_r_chunk(nb, ci):
            nsl = slice(nb * NBLK, (nb + 1) * NBLK)
            m0, nt = RCH[ci]
            rt = pz.tile([NBLK, 3, BANKW], dt.float32, tag="u", bufs=2,
                         name=f"r_{nb}_{ci}")
            for k in range(nt):
                nc.tensor.matmul(rt[:, k, 0:MT], f8ah[:, :, nsl],
                                 f8bh[:, :, m0 + k * MT: m0 + (k + 1) * MT],
                                 start=True, stop=True, perf_mode=DR)
            ehv = eh_t[nb][:, m0: m0 + nt * MT] \
                .rearrange("p (c w) -> p c w", w=MT)
            uh = rs.tile([NBLK, 1], dt.float32, tag=f"uh{ci}",
                         name=f"uh_{nb}_{ci}")
            nc.scalar.activation(ehv, rt[:, 0:nt, 0:MT], Act.Exp,
                                 accum_out=uh[:])
            return uh

        def emit_r_gfin(nb, uhp):
            ua = rs.tile([NBLK, 1], dt.float32, tag="ua", name=f"ua_{nb}")
            nc.vector.tensor_tensor(ua[:], uhp[0][:], uhp[1][:], Alu.add)
            nc.vector.tensor_tensor(ua[:], ua[:], uhp[2][:], Alu.add)
            gr = rs.tile([NBLK, 1], dt.float32, tag="gr", name=f"gr_{nb}")
            nc.vector.reciprocal(gr[:], ua[:])
            nc.vector.tensor_copy(g_t[nb][:], gr[:])
            if nb == NNB - 1:   # zero g on the 120 pad rows
                nc.vector.tensor_tensor(g_t[nb][:], g_t[nb][:],
                                        onesmask_t[:, nb:nb + 1], Alu.mult)

        # ---------------- phase F chunk ---------------------------------
        JPOS = {}
        JSWEEP = {}
        for _s, _js in enumerate(SWEEPJS):
            for _p, _jv in enumerate(_js):
                JPOS[_jv] = _p
                JSWEEP[_jv] = _s

        def emit_f_front(j, nbc, ev_eng, t_eng):
            jsl = slice(j * MT, (j + 1) * MT)
            nbs = [3 * nbc + k for k in range(3)]
            ft = pz.tile([NBLK, 3, BANKW], dt.float32, tag="u", bufs=2,
                         name=f"f_{j}_{nbc}")
            for kk, nb in enumerate(nbs):
                nsl = slice(nb * NBLK, (nb + 1) * NBLK)
                nc.tensor.matmul(ft[:, kk, 0:MT], f8av[:, :, nsl],
                                 f8bv[:, :, jsl], start=True, stop=True,
                                 perf_mode=DR)
            ev_i = fwkpool.tile([NBLK, 3, MT], dt.int16, tag="ev", bufs=6,
                                name=f"ev_{j}_{nbc}")
            evb = ev_i[:].bitcast(dt.bfloat16)
            if ev_eng == "act":
                nc.scalar.activation(evb, ft[:, :, 0:MT], Act.Exp)
            else:
                nc.vector.tensor_scalar(ev_i[:], ft[:, :, 0:MT], SA, SB,
                                        Alu.mult, Alu.add)
            t_t = fwkpool.tile([NBLK, 3, MT], dt.bfloat16, tag="t", bufs=6,
                               name=f"t_{j}_{nbc}")
            teng = nc.vector if t_eng == "dve" else nc.gpsimd
            for kk, nb in enumerate(nbs):
                teng.tensor_tensor(t_t[:, kk, :], eh_t[nb][:, jsl],
                                   ev_i[:, kk, :].bitcast(dt.bfloat16),
                                   Alu.mult)
            return (j, nbc, t_t, ev_i)

        def emit_f_mvs(front):
            j, nbc, t_t, ev_i = front
            jj = JPOS[j]
            nbs = [3 * nbc + k for k in range(3)]
            orow = oacc[32 * jj:32 * jj + 1, 0:MT]
            uvrow = uvacc[32 * jj:32 * jj + 1, 0:MT]
            for kk, nb in enumerate(nbs):
                nc.tensor.matmul(orow, g_t[nb][:], t_t[:, kk, :],
                                 start=(nb == 0), stop=(nb == NNB - 1),
                                 skip_group_check=True,
                                 tile_position=(0, 32 * jj))
                nc.tensor.matmul(uvrow, onesmask_t[:, nb:nb + 1],
                                 ev_i[:, kk, :].bitcast(dt.bfloat16),
                                 start=(nb == 0), stop=(nb == NNB - 1),
                                 skip_group_check=True,
                                 tile_position=(0, 32 * jj))

        def emit_sweep_drain(s):
            osb = fwkpool.tile([128, MT], dt.float32, tag="osb", bufs=1,
                               name=f"osb_{s}")
            uvsb = fwkpool.tile([128, MT], dt.float32, tag="uvsb", bufs=1,
                                name=f"uvsb_{s}")
            nc.vector.tensor_copy(osb[:], oacc[:, 0:MT])
            nc.vector.tensor_copy(uvsb[:], uvacc[:, 0:MT])
            ov = osb[:].rearrange("(q t) m -> q t m", t=32)[:, 0, :]
            uvv = uvsb[:].rearrange("(q t) m -> q t m", t=32)[:, 0, :]
            nc.sync.dma_start(o_out[s], ov)
            nc.sync.dma_start(uv_out[s], uvv)

        # ---------------- schedule --------------------------------------
        # F-chunk queue: sweep-major, then nbc-major within sweep
        fqueue = [(s, j, c) for s, js in enumerate(SWEEPJS)
                  for c in range(5) for j in js]
        f_next = 0
        pending = []       # emitted fronts awaiting their matvecs
        g_done = -1
        drained = -1       # last sweep whose accumulators were drained
        ev_alt = itertools.cycle(["act", "act", "dve"])
        ev_alt_r = itertools.cycle(["dve"])
        t_alt = itertools.cycle(["dve", "dve", "dve", "pool"])

        def f_ready():
            if f_next >= len(fqueue):
                return False
            s, j, c = fqueue[f_next]
            if g_done < 3 * c + 2:
                return False
            return True

        def maybe_drain():
            nonlocal drained
            s = drained + 1
            if s >= len(SWEEPJS):
                return
            n_done = sum(1 for i in range(f_next)
                         if fqueue[i][0] == s) - sum(1 for fr in pending
                                                    if JSWEEP[fr[0]] == s)
            if n_done == 5 * len(SWEEPJS[s]):
                emit_sweep_drain(s)
                drained = s

        def f_slot(during_r):
            nonlocal f_next
            lag = 5
            if (len(pending) >= lag + (1 if f_ready() else 0)) or \
                    (pending and not f_ready()):
                emit_f_mvs(pending.pop(0))
                maybe_drain()
            if f_ready():
                s, j, c = fqueue[f_next]
                eng = next(ev_alt_r) if during_r else next(ev_alt)
                pending.append(emit_f_front(j, c, eng, next(t_alt)))
                f_next += 1

        for nb in range(NNB):
            uhp = []
            for ci in range(3):
                uhp.append(emit_r_chunk(nb, ci))
                f_slot(during_r=True)
            emit_r_gfin(nb, uhp)
            g_done = nb
        while f_next < len(fqueue) or pending:
            f_slot(during_r=False)
        while drained < len(SWEEPJS) - 1:
            maybe_drain()

    nc.compile()
    return nc


def _get_nc(s):
    if s not in _CACHED:
        _CACHED[s] = _build(s)
    return _CACHED[s]


# ----------------------------------------------------------------------------
# Entry point
# ----------------------------------------------------------------------------

def kernel(**inputs):
    from concourse.bass_utils import run_bass_kernel_spmd

    in_maps = _host_prep(inputs)

    # One program for all 8 cores: the sample/row-half each core handles is
    # fully encoded in its host-built feature tiles.
    nc = _get_nc(0)
    last_err = None
    for attempt in range(3):
        try:
            r = run_bass_kernel_spmd(nc, in_maps, core_ids=list(range(NCORES)))
            break
        except Exception as e:  # transient NRT_EXEC_UNIT_UNRECOVERABLE wedges
            last_err = e
            import time
            time.sleep(10 * (attempt + 1))
    else:
        raise last_err
    results = r.results

    # host combine (exact)
    def _gather_m(arr):
        out = np.zeros(N, np.float64)
        for s, js in enumerate(SWEEPJS):
            for p, j in enumerate(js):
                out[j * MT:(j + 1) * MT] = arr[s, p].astype(np.float64)
        return out

    logs = np.zeros((B, N), np.float64)
    for b in range(B):
        r0, r1 = results[2 * b], results[2 * b + 1]
        O = _gather_m(r0["o_out"]) + _gather_m(r1["o_out"])
        uv = _gather_m(r0["uv_out"]) + _gather_m(r1["uv_out"])
        res_sum = O / uv
        logs[b] = np.log(res_sum + 1e-4)
    return np.float32(logs.mean())


# revision 45
# speedup vs baseline: 2.1542x; 1.0053x over previous
"""Trainium2 Bass kernel for nn_FMAPModelWarping (retrieval_knn).

The host does the cheap per-pixel prep (affine grids, bilinear taps, the
3x3x3->64 and 1x1 convs, 4-tap backward warp — ~1 GFLOP total, <4% of the
model) and ships fp8 feature maps. The 8 NeuronCores do the FLOP-heavy
part (~26.5 GFLOP): two 3600x3600x128 correlations per sample and the
bidirectional-softmax reduction, tiled flash-attention-style.

Sharding: core k = 2*b + s handles sample b (of 4) and row-half s of the
3600x3600 correlation matrices; partial column stats combine on the host.

Math restructure (exact):
  g[n] = 1/U_h[n],  res_sum[m] = O[m] / U_v[m],
  O[m] = sum_n g[n] * eh[n,m] * ev[n,m]
with U_h = rowsum(eh), U_v = colsum(ev), eh = exp(Mh), ev = exp(Mv).

Device structure: features live in a dual-plane [64, 2, n] fp8 layout so
the correlation matmuls run in DoubleRow perf mode (256-deep contraction,
0.5 cycles/row). Phase R computes exp(Mh) row-blocks (kept in SBUF) with
the row sums coming free from the activation engine's accumulator; phase F
(m-outer) recomputes exp(Mv), forms t = eh*ev, and accumulates O and U_v
via PSUM matvecs. R-chunks and F-chunks share a two-deep PSUM rotation and
are interleaved so the ACT exp stream, DVE/Pool elementwise work and PE
matmuls all overlap; part of the exp(Mv) field uses a Schraudolph bit-trick
exponential on DVE (its small relative noise cancels between O and U_v,
which consume the same ev values).
"""

import numpy as np

B, C_IN, H, W = 4, 3, 60, 60
HID, FEAT = 64, 128
N = H * W               # 3600
NCORES = 8
HALF = N // 2           # 1800 rows per core
NBLK = 128              # correlation row-block (partition dim)
NNB = 15                # row blocks per core (15*128 = 1920, rows padded)
NPAD = NNB * NBLK       # 1920
MT = 450                # m-tile width
N_MT = N // MT          # 8 m tiles
BANKW = 512             # fp32 elems per PSUM bank

# Schraudolph constants (bf16 target): i16 = rne(x*SA + SB); bits as bf16.
SA = 128.0 / float(np.log(2.0))
SB = 127.0 * 128.0 - 5.5 - 1.86   # -1.86 centers the measured +1% bias

# m-tile groups sharing the PSUM accumulator banks (4 rows at partition
# offsets 0/32/64/96 per bank; O and U_v each get one bank).
SWEEPJS = [(0, 1, 2, 3), (4, 5, 6, 7)]


# ----------------------------------------------------------------------------
# Host-side prep: exact reference semantics for grids / bilinear taps / rolls
# ----------------------------------------------------------------------------

def _affine_coords(theta2x3):
    """Pixel-space sample coords (x, y) for torch affine_grid+grid_sample
    (align_corners=False), shape [H, W] each."""
    xs = (2.0 * np.arange(W, dtype=np.float64) + 1.0) / W - 1.0
    ys = (2.0 * np.arange(H, dtype=np.float64) + 1.0) / H - 1.0
    gx, gy = np.meshgrid(xs, ys)           # gx[i,j]=xs[j], gy[i,j]=ys[i]
    t = theta2x3.astype(np.float64)
    cx = t[0, 0] * gx + t[0, 1] * gy + t[0, 2]
    cy = t[1, 0] * gx + t[1, 1] * gy + t[1, 2]
    px = (cx + 1.0) * W * 0.5 - 0.5
    py = (cy + 1.0) * H * 0.5 - 0.5
    return px, py


def _bilinear_sample_host(img, px, py):
    """img [C,H,W] float32, sample at (px,py) [H,W]; zeros padding.
    Mirrors reference grid_sample exactly."""
    x0 = np.floor(px); y0 = np.floor(py)
    wx1 = (px - x0); wx0 = 1.0 - wx1
    wy1 = (py - y0); wy0 = 1.0 - wy1
    out = np.zeros((img.shape[0],) + px.shape, np.float64)
    flat = img.reshape(img.shape[0], -1).astype(np.float64)
    for ix, iy, wt in ((x0, y0, wx0 * wy0), (x0 + 1, y0, wx1 * wy0),
                       (x0, y0 + 1, wx0 * wy1), (x0 + 1, y0 + 1, wx1 * wy1)):
        valid = (ix >= 0) & (ix < W) & (iy >= 0) & (iy < H)
        ii = np.clip(ix, 0, W - 1).astype(np.int64)
        jj = np.clip(iy, 0, H - 1).astype(np.int64)
        v = flat[:, (jj * W + ii).ravel()].reshape(out.shape)
        out += v * (wt * valid)[None]
    return out.astype(np.float32)


def _back_taps(theta2x3, u, v):
    """Tap indices/weights for grid_sample(y, grid(Bm)) composed with the
    inverse roll. Returns idx [4,3600] int (in-range), wt [4,3600] f32."""
    px, py = _affine_coords(theta2x3)
    ii = np.arange(H)[:, None]; jj = np.arange(W)[None, :]
    qi = (ii - u) % H; qj = (jj - v) % W
    xs = px[qi, qj].ravel(); ys = py[qi, qj].ravel()
    x0 = np.floor(xs); y0 = np.floor(ys)
    fx = xs - x0; fy = ys - y0
    idxs, wts = [], []
    for ix, iy, wt in ((x0, y0, (1 - fx) * (1 - fy)), (x0 + 1, y0, fx * (1 - fy)),
                       (x0, y0 + 1, (1 - fx) * fy), (x0 + 1, y0 + 1, fx * fy)):
        valid = (ix >= 0) & (ix < W) & (iy >= 0) & (iy < H)
        cii = np.clip(ix, 0, W - 1).astype(np.int64)
        cjj = np.clip(iy, 0, H - 1).astype(np.int64)
        idxs.append(cjj * W + cii)
        wts.append((wt * valid).astype(np.float32))
    return np.stack(idxs), np.stack(wts)


def _host_prep(inputs):
    """Build the 8 per-core device input dicts (fp8 dual-plane features)."""
    import ml_dtypes
    x_a = np.asarray(inputs["input_a"], np.float32)
    x_b = np.asarray(inputs["input_b"], np.float32)
    w1 = np.asarray(inputs["w1"], np.float32)
    b1 = np.asarray(inputs["b1"], np.float32)
    w2 = np.asarray(inputs["w2"], np.float32)
    b2 = np.asarray(inputs["b2"], np.float32)
    noise = np.asarray(inputs["noise"], np.float32)
    u_roll = np.asarray(inputs["u_roll"])
    v_roll = np.asarray(inputs["v_roll"])
    swap = np.asarray(inputs["swap"])

    w1mat = w1.reshape(HID, C_IN * 9)                  # [64, 27]
    w2mat = w2.reshape(FEAT, HID)                      # [128, 64]

    eye = np.eye(3, dtype=np.float64)
    mask = np.array([[1., 1., 1.], [1., 1., 1.], [0., 0., 0.]])

    # F[wrp][b]: warped feature map [FEAT, N] float32 (exact reference math;
    # the 1x1 conv2 commutes with the backward spatial gather)
    F = np.zeros((4, B, FEAT, N), np.float32)
    for wrp in range(4):
        sw = int(swap[wrp]) == 1
        for b in range(B):
            fwd = eye + 0.05 * noise[wrp, b].astype(np.float64) * mask
            bwd = np.linalg.inv(fwd)
            A_ = bwd if sw else fwd
            Bm = fwd if sw else bwd
            u = int(u_roll[wrp, b]); v = int(v_roll[wrp, b])
            img = x_a[b] if wrp in (0, 2) else x_b[b]
            x_r = np.roll(np.roll(img, -u, axis=1), -v, axis=2)
            px, py = _affine_coords(np.asarray(A_)[:2])
            xw = _bilinear_sample_host(x_r, px, py)       # [3,60,60]
            # im2col, zero-pad SAME, k = c*9 + ky*3 + kx
            pad = np.zeros((C_IN, H + 2, W + 2), np.float32)
            pad[:, 1:-1, 1:-1] = xw
            X1 = np.zeros((C_IN * 9, N), np.float32)
            k = 0
            for c in range(C_IN):
                for ky in range(3):
                    for kx in range(3):
                        X1[k] = pad[c, ky:ky + H, kx:kx + W].ravel()
                        k += 1
            y1 = np.maximum(w1mat @ X1 + b1[:, None], 0.0)   # [64, N]
            y2 = w2mat @ y1 + b2[:, None]                    # [128, N]
            idx, wt = _back_taps(np.asarray(Bm)[:2], u, v)
            Fw = np.zeros((FEAT, N), np.float32)
            for tap in range(4):
                Fw += y2[:, idx[tap]] * wt[tap][None, :]
            F[wrp, b] = Fw

    F8 = F.astype(ml_dtypes.float8_e4m3fn)

    def dual_plane(feat, cols):
        """[FEAT, n] -> [64, 2, cols] (zero-padded)."""
        out = np.zeros((64, 2, cols), ml_dtypes.float8_e4m3fn)
        n = feat.shape[1]
        out[:, 0, :n] = feat[0:64]
        out[:, 1, :n] = feat[64:128]
        return out

    # U_v matvec stationaries: ones, except block 14 masks the 120 pad rows
    onesmask = np.ones((128, NNB), np.float32)
    onesmask[8:, NNB - 1] = 0.0

    in_maps = []
    for core in range(NCORES):
        b = core // 2
        s = core % 2
        n0 = s * HALF
        in_maps.append({
            "f8ah_in": dual_plane(F8[0, b][:, n0:n0 + HALF], NPAD),
            "f8av_in": dual_plane(F8[2, b][:, n0:n0 + HALF], NPAD),
            "f8bh_in": dual_plane(F8[1, b], N),
            "f8bv_in": dual_plane(F8[3, b], N),
            "onesmask_in": onesmask.astype(ml_dtypes.bfloat16),
        })
    return in_maps


# ----------------------------------------------------------------------------
# Device kernel builder
# ----------------------------------------------------------------------------

_CACHED = {}


def _build(core_half):
    """Build the Bacc module (one NEFF shared by all 8 cores; each core's
    sample/row-half is fully encoded in its host-built feature tiles)."""
    import concourse.bacc as bacc_mod
    import concourse.mybir as mybir
    from concourse.tile import TileContext
    from contextlib import ExitStack
    import itertools

    dt = mybir.dt
    Alu = mybir.AluOpType
    Act = mybir.ActivationFunctionType
    DR = mybir.MatmulPerfMode.DoubleRow

    nc = bacc_mod.Bacc("TRN2", target_bir_lowering=False)

    f8ah_in = nc.dram_tensor("f8ah_in", [64, 2, NPAD], dt.float8e4, kind="ExternalInput")
    f8av_in = nc.dram_tensor("f8av_in", [64, 2, NPAD], dt.float8e4, kind="ExternalInput")
    f8bh_in = nc.dram_tensor("f8bh_in", [64, 2, N], dt.float8e4, kind="ExternalInput")
    f8bv_in = nc.dram_tensor("f8bv_in", [64, 2, N], dt.float8e4, kind="ExternalInput")
    onesmask_in = nc.dram_tensor("onesmask_in", [FEAT, NNB], dt.bfloat16, kind="ExternalInput")

    o_out = nc.dram_tensor("o_out", [2, 4, MT], dt.float32, kind="ExternalOutput")
    uv_out = nc.dram_tensor("uv_out", [2, 4, MT], dt.float32, kind="ExternalOutput")

    with ExitStack() as ctx:
        tc = ctx.enter_context(TileContext(nc))

        const = ctx.enter_context(tc.tile_pool(name="const", bufs=1))
        onesmask_t = const.tile([FEAT, NNB], dt.bfloat16)
        nc.sync.dma_start(onesmask_t[:], onesmask_in[:])

        fpool = ctx.enter_context(tc.tile_pool(name="feat", bufs=1))
        f8ah = fpool.tile([64, 2, NPAD], dt.float8e4, name="f8ah")
        f8av = fpool.tile([64, 2, NPAD], dt.float8e4, name="f8av")
        f8bh = fpool.tile([64, 2, N], dt.float8e4, name="f8bh")
        f8bv = fpool.tile([64, 2, N], dt.float8e4, name="f8bv")
        nc.sync.dma_start(f8ah[:], f8ah_in[:])
        nc.sync.dma_start(f8bh[:], f8bh_in[:])
        nc.sync.dma_start(f8av[:], f8av_in[:])
        nc.sync.dma_start(f8bv[:], f8bv_in[:])

        stat = ctx.enter_context(tc.tile_pool(name="stat", bufs=1))
        eh_t = [stat.tile([NBLK, N], dt.bfloat16, name=f"eh{_nb}")
                for _nb in range(NNB)]
        g_t = [stat.tile([NBLK, 1], dt.bfloat16, name=f"g{_nb}")
               for _nb in range(NNB)]

        rs = ctx.enter_context(tc.tile_pool(name="rsmall", bufs=10))

        # Single PSUM pool, bank budget 8: tag "u" (2 x 3 banks, a unified
        # two-deep rotation shared by R chunks and F chunks — consecutive
        # chunks land in alternating buffers so each chunk's matmuls overlap
        # the previous chunk's exp), plus two accumulator banks (4 rows each
        # at partition offsets 0/32/64/96).
        pz = ctx.enter_context(tc.tile_pool(name="pz", bufs=1, space="PSUM"))
        fwkpool = ctx.enter_context(tc.tile_pool(name="fwkpool", bufs=1))

        oacc = pz.tile([NBLK, BANKW], dt.float32, tag="accA", name="oacc")
        uvacc = pz.tile([NBLK, BANKW], dt.float32, tag="accB", name="uvacc")

        # R chunking: m ranges as (start, n_tiles) with 450-wide tiles
        RCH = [(0, 3), (1350, 3), (2700, 2)]

        def emit_r_chunk(nb, ci):
            nsl = slice(nb * NBLK, (nb + 1) * NBLK)
            m0, nt = RCH[ci]
            rt = pz.tile([NBLK, 3, BANKW], dt.float32, tag="u", bufs=2,
                         name=f"r_{nb}_{ci}")
            for k in range(nt):
                nc.tensor.matmul(rt[:, k, 0:MT], f8ah[:, :, nsl],
                                 f8bh[:, :, m0 + k * MT: m0 + (k + 1) * MT],
                                 start=True, stop=True, perf_mode=DR)
            ehv = eh_t[nb][:, m0: m0 + nt * MT] \
                .rearrange("p (c w) -> p c w", w=MT)
            uh = rs.tile([NBLK, 1], dt.float32, tag=f"uh{ci}",
                         name=f"uh_{nb}_{ci}")
            nc.scalar.activation(ehv, rt[:, 0:nt, 0:MT], Act.Exp,
                                 accum_out=uh[:])
            return uh

        def emit_r_gfin(nb, uhp):
            ua = rs.tile([NBLK, 1], dt.float32, tag="ua", name=f"ua_{nb}")
            nc.vector.tensor_tensor(ua[:], uhp[0][:], uhp[1][:], Alu.add)
            nc.vector.tensor_tensor(ua[:], ua[:], uhp[2][:], Alu.add)
            gr = rs.tile([NBLK, 1], dt.float32, tag="gr", name=f"gr_{nb}")
            nc.vector.reciprocal(gr[:], ua[:])
            nc.vector.tensor_copy(g_t[nb][:], gr[:])
            if nb == NNB - 1:   # zero g on the 120 pad rows
                nc.vector.tensor_tensor(g_t[nb][:], g_t[nb][:],
                                        onesmask_t[:, nb:nb + 1], Alu.mult)

        # ---------------- phase F chunk ---------------------------------
        JPOS = {}
        JSWEEP = {}
        for _s, _js in enumerate(SWEEPJS):
            for _p, _jv in enumerate(_js):
                JPOS[_jv] = _p
                JSWEEP[_jv] = _s

        def emit_f_front(j, nbc, ev_eng, t_eng):
            jsl = slice(j * MT, (j + 1) * MT)
            nbs = [3 * nbc + k for k in range(3)]
            ft = pz.tile([NBLK, 3, BANKW], dt.float32, tag="u", bufs=2,
                         name=f"f_{j}_{nbc}")
            for kk, nb in enumerate(nbs):
                nsl = slice(nb * NBLK, (nb + 1) * NBLK)
                nc.tensor.matmul(ft[:, kk, 0:MT], f8av[:, :, nsl],
                                 f8bv[:, :, jsl], start=True, stop=True,
                                 perf_mode=DR)
            ev_i = fwkpool.tile([NBLK, 3, MT], dt.int16, tag="ev", bufs=6,
                                name=f"ev_{j}_{nbc}")
            evb = ev_i[:].bitcast(dt.bfloat16)
            if ev_eng == "act":
                nc.scalar.activation(evb, ft[:, :, 0:MT], Act.Exp)
            else:
                nc.vector.tensor_scalar(ev_i[:], ft[:, :, 0:MT], SA, SB,
                                        Alu.mult, Alu.add)
            t_t = fwkpool.tile([NBLK, 3, MT], dt.bfloat16, tag="t", bufs=6,
                               name=f"t_{j}_{nbc}")
            teng = nc.vector if t_eng == "dve" else nc.gpsimd
            for kk, nb in enumerate(nbs):
                teng.tensor_tensor(t_t[:, kk, :], eh_t[nb][:, jsl],
                                   ev_i[:, kk, :].bitcast(dt.bfloat16),
                                   Alu.mult)
            return (j, nbc, t_t, ev_i)

        def emit_f_mvs(front):
            j, nbc, t_t, ev_i = front
            jj = JPOS[j]
            nbs = [3 * nbc + k for k in range(3)]
            orow = oacc[32 * jj:32 * jj + 1, 0:MT]
            uvrow = uvacc[32 * jj:32 * jj + 1, 0:MT]
            for kk, nb in enumerate(nbs):
                nc.tensor.matmul(orow, g_t[nb][:], t_t[:, kk, :],
                                 start=(nb == 0), stop=(nb == NNB - 1),
                                 skip_group_check=True,
                                 tile_position=(0, 32 * jj))
                nc.tensor.matmul(uvrow, onesmask_t[:, nb:nb + 1],
                                 ev_i[:, kk, :].bitcast(dt.bfloat16),
                                 start=(nb == 0), stop=(nb == NNB - 1),
                                 skip_group_check=True,
                                 tile_position=(0, 32 * jj))

        def emit_sweep_drain(s):
            osb = fwkpool.tile([128, MT], dt.float32, tag="osb", bufs=1,
                               name=f"osb_{s}")
            uvsb = fwkpool.tile([128, MT], dt.float32, tag="uvsb", bufs=1,
                                name=f"uvsb_{s}")
            nc.vector.tensor_copy(osb[:], oacc[:, 0:MT])
            nc.vector.tensor_copy(uvsb[:], uvacc[:, 0:MT])
            ov = osb[:].rearrange("(q t) m -> q t m", t=32)[:, 0, :]
            uvv = uvsb[:].rearrange("(q t) m -> q t m", t=32)[:, 0, :]
            nc.sync.dma_start(o_out[s], ov)
            nc.sync.dma_start(uv_out[s], uvv)

        # ---------------- schedule --------------------------------------
        # F-chunk queue: sweep-major, then nbc-major within sweep
        fqueue = [(s, j, c) for s, js in enumerate(SWEEPJS)
                  for c in range(5) for j in js]
        f_next = 0
        pending = []       # emitted fronts awaiting their matvecs
        g_done = -1
        drained = -1       # last sweep whose accumulators were drained
        ev_alt = itertools.cycle(["act", "act", "dve"])
        ev_alt_r = itertools.cycle(["dve"])
        t_alt = itertools.cycle(["dve", "dve", "dve", "pool"])

        def f_ready():
            if f_next >= len(fqueue):
                return False
            s, j, c = fqueue[f_next]
            if g_done < 3 * c + 2:
                return False
            return True

        def maybe_drain():
            nonlocal drained
            s = drained + 1
            if s >= len(SWEEPJS):
                return
            n_done = sum(1 for i in range(f_next)
                         if fqueue[i][0] == s) - sum(1 for fr in pending
                                                    if JSWEEP[fr[0]] == s)
            if n_done == 5 * len(SWEEPJS[s]):
                emit_sweep_drain(s)
                drained = s

        def f_slot(during_r):
            nonlocal f_next
            lag = 4
            if (len(pending) >= lag + (1 if f_ready() else 0)) or \
                    (pending and not f_ready()):
                emit_f_mvs(pending.pop(0))
                maybe_drain()
            if f_ready():
                s, j, c = fqueue[f_next]
                eng = next(ev_alt_r) if during_r else next(ev_alt)
                pending.append(emit_f_front(j, c, eng, next(t_alt)))
                f_next += 1

        for nb in range(NNB):
            uhp = []
            for ci in range(3):
                uhp.append(emit_r_chunk(nb, ci))
                f_slot(during_r=True)
            emit_r_gfin(nb, uhp)
            g_done = nb
        while f_next < len(fqueue) or pending:
            f_slot(during_r=False)
        while drained < len(SWEEPJS) - 1:
            maybe_drain()

    nc.compile()
    return nc


def _get_nc(s):
    if s not in _CACHED:
        _CACHED[s] = _build(s)
    return _CACHED[s]


# ----------------------------------------------------------------------------
# Entry point
# ----------------------------------------------------------------------------

def kernel(**inputs):
    from concourse.bass_utils import run_bass_kernel_spmd

    in_maps = _host_prep(inputs)

    # One program for all 8 cores: the sample/row-half each core handles is
    # fully encoded in its host-built feature tiles.
    nc = _get_nc(0)
    last_err = None
    for attempt in range(3):
        try:
            r = run_bass_kernel_spmd(nc, in_maps, core_ids=list(range(NCORES)))
            break
        except Exception as e:  # transient NRT_EXEC_UNIT_UNRECOVERABLE wedges
            last_err = e
            import time
            time.sleep(10 * (attempt + 1))
    else:
        raise last_err
    results = r.results

    # host combine (exact)
    def _gather_m(arr):
        out = np.zeros(N, np.float64)
        for s, js in enumerate(SWEEPJS):
            for p, j in enumerate(js):
                out[j * MT:(j + 1) * MT] = arr[s, p].astype(np.float64)
        return out

    logs = np.zeros((B, N), np.float64)
    for b in range(B):
        r0, r1 = results[2 * b], results[2 * b + 1]
        O = _gather_m(r0["o_out"]) + _gather_m(r1["o_out"])
        uv = _gather_m(r0["uv_out"]) + _gather_m(r1["uv_out"])
        res_sum = O / uv
        logs[b] = np.log(res_sum + 1e-4)
    return np.float32(logs.mean())
